# revision 1
# baseline (speedup 1.0000x reference)
"""Trainium2 Bass kernel for nn_DiTXMoEBlock (DiT block: adaLN + self-attn +
gated cross-attn + top-2-of-8 MoE FFN + shared expert).

Strategy (8 NeuronCores, full inputs in / full output out):
- Pass A (data-parallel, 512 query tokens per core = half a batch):
  everything up to the MoE router. All heavy matmuls run as fp8e4m3
  DoubleRow (2 k-planes per PE pass). The adaLN scales are folded into
  the weights on the host (per batch); the per-token LN shift enters via
  a rank-2 correction matmul [u;w0]^T[b;1] accumulated into each psum.
  V tiles are token-major with 64 ones-columns per head so the AV matmul
  also produces the softmax row-sum broadcast across 64 psum rows (free
  denominator). Self-attention score/exp streams (ACT-bound) are
  interleaved with the V/K2/V2 projections and lag-2 AV matmuls to keep
  the tensor engine busy. Router logits are computed on device from the
  pre-fp8 xhat3 (token-major, 32-wide padded) so routing precision is
  f32r-grade; elementwise work is spread across DVE/ACT/GPSIMD.
- Host: softmax + top-2 in fp32 from device logits (+ per-batch
  sh_mlp@W_router + tcond constants); near-tie tokens (margin < 1e-2)
  are re-derived in full fp32 so routing matches the reference exactly.
- Pass B (bin-packed expert FFN, fp8 DoubleRow): 3 bins/core with
  compile-time sizes (smallest feasible config chosen at runtime via a
  backtracking packer; each bin holds one expert's tokens, shared expert
  fills the slack). h2 arrives fp8 partition-major straight from pass A;
  W1/W2 stream per bin at fp8 (x64 pre-scale); gelu runs as two wide
  12-plane ACT ops per 128-token chunk; the y eviction (x 1/64, bf16)
  runs on DVE.
- Host: weighted scatter-add combine + final residual.
"""

import numpy as np

try:
    import concourse.bacc as bacc
except ImportError:  # fall back to the repo checkout location
    import sys
    sys.path.insert(0, "/opt/trn_rl_repo")
    import concourse.bacc as bacc

import ml_dtypes
import concourse.bass as bass
import concourse.mybir as mybir
from concourse.bass_utils import run_bass_kernel_spmd
from concourse.tile import TileContext

AF = mybir.ActivationFunctionType
ALU = mybir.AluOpType
f32 = mybir.dt.float32
f32r = mybir.dt.float32r
bf16 = mybir.dt.bfloat16

B, N, L, C, H, E, TOPK, F = 4, 1024, 512, 768, 12, 8, 2, 3072
D = C // H          # 64
P = 128             # partitions
T = 512             # tokens per core in pass A (half a batch)
CT = C // P         # 6 C-tiles
FT = F // P         # 24 F-tiles
NKT = N // P        # 8 key tiles (self-attention, full batch seq)
LKT = L // P        # 4 key tiles (cross-attention context)
EPS = 1e-5
NSLOT = 4           # expert-chunk slots per core in pass B
CAP = 512           # tokens per chunk slot
VW = 128            # per-head width in v_tm layout (64 v cols + 64 ones cols
                    # whose AV product replicates the softmax row-sum)

_CACHE = {}
LAST_A_KEY = ("a", False)   # pass-A variant used on the last kernel() call
LAST_B_KEY = None           # pass-B variant used on the last kernel() call


def prep_w(Wmat):
    """[K, M] fp32 -> [P, (K//P)*M] fp8, scaled by FP8S, partition-major:
    w[p, k, m] = FP8S * W[k*P+p, m]."""
    Kd, Md = Wmat.shape
    return np.ascontiguousarray(
        (Wmat * FP8S).reshape(Kd // P, P, Md).transpose(1, 0, 2)
        .reshape(P, (Kd // P) * Md).astype(ml_dtypes.float8_e4m3fn))


# --------------------------------------------------------------------------
# Pass A kernel builder (fp8 DoubleRow linears, LN scale folded into weights)
# --------------------------------------------------------------------------

# rank-2 row indices in the urows input (u = col-sums of scaled W, w0 = W^T sh)
NPROJ = 9           # q, k, v, qq, qg, k2, v2, psa, pca
(PROJ_Q, PROJ_K, PROJ_V, PROJ_QQ, PROJ_QG, PROJ_K2, PROJ_V2,
 PROJ_PSA, PROJ_PCA) = range(NPROJ)

def _build_pass_a(has_pbias=False):
    fp8 = mybir.dt.float8e4
    DR = mybir.MatmulPerfMode.DoubleRow
    nc = bacc.Bacc("TRN2", target_bir_lowering=False, debug=False, num_devices=8)

    din = {}
    for nm, shape, dt_ in [
        ("xT", [C, T], f32r), ("xoT", [C, T], bf16), ("cT", [C, T], bf16),
        ("WQ8", [P, CT * C], fp8), ("WK8", [P, CT * C], fp8),
        ("WV8", [P, CT * C], fp8), ("WQQ8", [P, CT * C], fp8),
        ("WQG8", [P, CT * C], fp8), ("WK28", [P, CT * C], fp8),
        ("WV28", [P, CT * C], fp8), ("WP8", [P, CT * C], fp8),
        ("WP28", [P, CT * C], fp8),
        ("WR", [P, CT * 32], bf16),
        ("urows", [2, NPROJ * C], f32r),
        ("cols", [P, 3 * CT], f32),
        ("onesr", [1, T], f32r), ("onesc", [P, 1], f32r),
        ("vinit", [P, 2 * H * VW], fp8),
    ]:
        din[nm] = nc.dram_tensor(nm, shape, dt_, kind="ExternalInput")
    x3_out = nc.dram_tensor("x3T", [C, T], f32r, kind="ExternalOutput")
    h2_out = nc.dram_tensor("h2pm", [P, CT * T], fp8, kind="ExternalOutput")
    lg_out = nc.dram_tensor("lgT", [P, 4 * 32], f32, kind="ExternalOutput")

    with TileContext(nc) as tc, \
         nc.allow_low_precision("fp8/f32r rounding of matmul operands is intended"):
        with tc.tile_pool(name="acts", bufs=1) as acts, \
             tc.tile_pool(name="wpool", bufs=3) as wpool, \
             tc.tile_pool(name="vec", bufs=1) as vecp, \
             tc.tile_pool(name="sq", bufs=4) as sqp, \
             tc.tile_pool(name="exps", bufs=12) as expp, \
             tc.tile_pool(name="bca", bufs=3) as bcap, \
             tc.tile_pool(name="ps_lin", bufs=2, space="PSUM") as ps_lin, \
             tc.tile_pool(name="ps_sc", bufs=2, space="PSUM") as ps_sc, \
             tc.tile_pool(name="ps_misc", bufs=2, space="PSUM") as ps_misc:

            # ---------- constants ----------
            ones_col = vecp.tile([P, 1], f32r, tag="ones_col")
            nc.sync.dma_start(ones_col[:, :], din["onesc"][:, :])
            ones_col_b = vecp.tile([P, 1], bf16, tag="ones_col_b")
            nc.vector.memset(ones_col_b[:, :], 1.0)

            ones_row = vecp.tile([1, T], f32r, tag="ones_row")
            nc.sync.dma_start(ones_row[:, :], din["onesr"][:, :])
            eps_t = vecp.tile([1, 1], f32, tag="eps")
            nc.vector.memset(eps_t[:, :], EPS)
            cols = vecp.tile([P, 3 * CT], f32, tag="cols")
            nc.sync.dma_start(cols[:, :], din["cols"][:, :])
            c_sc1f, c_shf = cols[:, 0:CT], cols[:, CT:2 * CT]
            c_g64 = cols[:, 2 * CT:3 * CT]          # g_msa / FP8S
            urows = vecp.tile([2, NPROJ * C], f32r, tag="urows")
            nc.sync.dma_start(urows[:, :], din["urows"][:, :])

            # ---------- activations ----------
            def load_fm(name, tag, dt_=f32r):
                ts = []
                for i in range(CT):
                    t_ = acts.tile([P, T], dt_, tag=f"{tag}{i}")
                    nc.sync.dma_start(t_[:, :], din[name][i * P:(i + 1) * P, :])
                    ts.append(t_)
                return ts

            x_t = load_fm("xT", "x")

            def load_w(name):
                w = wpool.tile([P, CT, C], fp8, tag="wblk")
                nc.sync.dma_start(
                    w[:, :, :], din[name].rearrange("p (k c) -> p k c", k=CT))
                return w

            # ---------- LayerNorm stats ----------
            def ln_stats(src_tiles, sq_engine="pool", oc_=None):
                """Returns bcA [P,T] f32 PSUM tile (rstd broadcast) and
                brows [2,T] f32r = (b = -mean*rstd ; ones)."""
                oc_ = ones_col if oc_ is None else oc_
                st_x = ps_misc.tile([P, T], f32, tag="misc")
                for i in range(CT):
                    nc.tensor.matmul(st_x[0:1, :], oc_[:, :],
                                     src_tiles[i][:, :],
                                     start=(i == 0), stop=(i == CT - 1))
                st_xx = ps_misc.tile([P, T], f32, tag="misc")
                for i in range(CT):
                    sq = sqp.tile([P, T], bf16, tag="sq")
                    # alternate engines: halves the serial square-chain latency
                    if (i % 2 == 0) == (sq_engine == "pool"):
                        nc.gpsimd.tensor_mul(sq[:, :], src_tiles[i][:, :],
                                             src_tiles[i][:, :])
                    else:
                        nc.vector.tensor_mul(sq[:, :], src_tiles[i][:, :],
                                             src_tiles[i][:, :])
                    nc.tensor.matmul(st_xx[0:1, :], ones_col_b[:, :], sq[:, :],
                                     start=(i == 0), stop=(i == CT - 1))
                m2 = vecp.tile([1, T], f32, tag="m2", bufs=2)
                nc.scalar.square(m2[:, :], st_x[0:1, :])
                varp = vecp.tile([1, T], f32, tag="varp", bufs=2)
                nc.vector.scalar_tensor_tensor(varp[:, :], m2[:, :], -1.0 / C,
                                               st_xx[0:1, :], ALU.mult, ALU.add)
                sd = vecp.tile([1, T], f32, tag="sd", bufs=2)
                nc.scalar.activation(sd[:, :], varp[:, :], AF.Sqrt,
                                     bias=eps_t[:, 0:1], scale=1.0 / C)
                a = vecp.tile([1, T], f32r, tag="a", bufs=2)
                nc.vector.reciprocal(a[:, :], sd[:, :])
                brows = vecp.tile([2, T], f32r, tag="brows", bufs=4)
                nc.sync.dma_start(brows[1:2, :], din["onesr"][:, :])
                nc.vector.scalar_tensor_tensor(brows[0:1, :], st_x[0:1, :],
                                               -1.0 / C, a[:, :],
                                               ALU.mult, ALU.mult)
                bcA = ps_misc.tile([P, T], f32, tag="misc")
                nc.tensor.matmul(bcA[:, :], ones_row[:, 0:P], a[:, :],
                                 start=True, stop=True)
                return bcA, brows

            def ln_xhat_dve(src_tiles, bcA, tag):
                bcs = bcap.tile([P, T], f32, tag="bcs")
                nc.vector.tensor_copy(bcs[:, :], bcA[:, :])
                xh = acts.tile([P, CT, T], fp8, tag=tag)
                for i in range(CT):
                    if i % 2 == 0:
                        nc.vector.tensor_mul(xh[:, i, :], src_tiles[i][:, :],
                                             bcA[:, :])
                    else:
                        nc.gpsimd.tensor_mul(xh[:, i, :], src_tiles[i][:, :],
                                             bcs[:, :])
                return xh

            def ln_xhat_pool(src_tiles, bcA, tag):
                # even planes on DVE (psum bcA), odd planes on Pool (sbuf
                # copy) -- halves the serial latency of the apply chain
                bcs = bcap.tile([P, T], f32, tag="bcs")
                nc.vector.tensor_copy(bcs[:, :], bcA[:, :])
                xh = acts.tile([P, CT, T], fp8, tag=tag)
                for i in range(CT):
                    if i % 2 == 0:
                        nc.vector.tensor_mul(xh[:, i, :], src_tiles[i][:, :],
                                             bcA[:, :])
                    else:
                        nc.gpsimd.tensor_mul(xh[:, i, :], src_tiles[i][:, :],
                                             bcs[:, :])
                return xh

            # ---------- fp8 DoubleRow linear (feature-major out) ----------
            HC = T // 2    # 256-token matmul chunks

            def linear_oi(w, xh, brows, uix, evict, oi):
                    ps = ps_lin.tile([P, T], f32, tag="lin")
                    for hcb in range(2):
                        cs = slice(hcb * HC, (hcb + 1) * HC)
                        for pi in range(CT // 2):
                            nc.tensor.matmul(
                                ps[:, cs], w[:, 2 * pi:2 * pi + 2,
                                             oi * P:(oi + 1) * P],
                                xh[:, 2 * pi:2 * pi + 2, cs],
                                start=(pi == 0),
                                stop=(pi == CT // 2 - 1 and uix is None),
                                perf_mode=DR)
                        if uix is not None:
                            nc.tensor.matmul(
                                ps[:, cs],
                                urows[0:2,
                                      uix * C + oi * P:uix * C + (oi + 1) * P],
                                brows[0:2, cs], start=False, stop=True)
                    evict(oi, ps)

            def linear(w, xh, brows, uix, evict):
                for oi in range(CT):
                    linear_oi(w, xh, brows, uix, evict, oi)

            # ---------- LN1 + Q/K (xo/c stats stream in behind) ----------
            wq = load_w("WQ8")
            wk = load_w("WK8")
            bcA1, brows1 = ln_stats(x_t, sq_engine="dve")
            xh1 = ln_xhat_dve(x_t, bcA1, "xh1")

            q_t = [acts.tile([P, T], bf16, tag=f"q{i}", name=f"q{i}")
                   for i in range(CT)]

            def evict_q(oi, ps):
                nc.vector.tensor_scalar_mul(q_t[oi][:, :], ps[:, :], 1.0 / FP8S)
            linear(wq, xh1, brows1, PROJ_Q, evict_q)

            k_t = [acts.tile([P, N], bf16, tag=f"k{i}", name=f"k{i}")
                   for i in range(CT)]

            def evict_k0(oi, ps):
                nc.vector.tensor_scalar_mul(k_t[oi][:, 0:T], ps[:, :],
                                            1.0 / FP8S)
            linear(wk, xh1, brows1, PROJ_K, evict_k0)

            xo_t = load_fm("xoT", "xo", bf16)
            bcA1o, brows1o = ln_stats(xo_t, oc_=ones_col_b)
            xh1o = ln_xhat_pool(xo_t, bcA1o, "xh1o")

            def evict_k1(oi, ps):
                nc.vector.tensor_scalar_mul(k_t[oi][:, T:N], ps[:, :],
                                            1.0 / FP8S)
            linear(wk, xh1o, brows1o, PROJ_K, evict_k1)

            c_t = load_fm("cT", "c", bf16)
            bcAc, browsc = ln_stats(c_t, oc_=ones_col_b)
            xhc = ln_xhat_pool(c_t, bcAc, "xhc")

            # ---------- V tiles (token-major interleaved fp8 pairs) ---------
            wv = load_w("WV8")
            v_tm = []
            for ktp in range(NKT // 2):
                vt = acts.tile([P, 2, H * VW], fp8, tag=f"vtm{ktp}",
                               name=f"vtm{ktp}")
                v_tm.append(vt)
                nc.sync.dma_start(
                    vt[:, :, :],
                    din["vinit"].rearrange("p (j w) -> p j w", j=2))

            def v_chunk(wv_, xh_, br_, uix, v_tiles, half, tj, oc):
                """One [128tok, 256feat] chunk of the V projection."""
                kt = half * 4 + tj
                ktp, par = kt // 2, kt % 2
                ps = ps_lin.tile([P, T], f32, tag="lin")
                for pi in range(CT // 2):
                    nc.tensor.matmul(
                        ps[:, 0:256],
                        xh_[:, 2 * pi:2 * pi + 2, tj * P:(tj + 1) * P],
                        wv_[:, 2 * pi:2 * pi + 2, oc * 256:(oc + 1) * 256],
                        start=(pi == 0), stop=False, perf_mode=DR)
                nc.tensor.matmul(
                    ps[:, 0:256], br_[0:2, tj * P:(tj + 1) * P],
                    urows[0:2, uix * C + oc * 256:uix * C + (oc + 1) * 256],
                    start=False, stop=True)
                vb = v_tiles[ktp][:, par, :]
                dst = bass.AP(vb.tensor, vb.offset + oc * 4 * VW,
                              [list(vb.ap[0]), [VW, 4], [1, 64]])
                nc.vector.tensor_scalar_mul(
                    dst, ps[:, 0:256].rearrange("p (h d) -> p h d", h=4),
                    1.0 / FP8S)

            # ---------- attention helpers (VW=128: ones cols -> rowsum) -----
            def attn_scores(q_tiles, k_tiles, h, nkt):
                th, ro = h // 2, 64 * (h % 2)
                ex_tiles = []
                for ktp in range(nkt // 2):
                    sps = ps_sc.tile([P, 2, T], f32, tag="score")
                    for par in range(2):
                        kt = 2 * ktp + par
                        nc.tensor.matmul(
                            sps[:, par, :],
                            k_tiles[th][ro:ro + 64, kt * P:(kt + 1) * P],
                            q_tiles[th][ro:ro + 64, :],
                            start=True, stop=True)
                    ex = expp.tile([P, 2, T], fp8, tag="exp")
                    nc.scalar.activation(ex[:, :, :], sps[:, :, :], AF.Exp,
                                         scale=float(D ** -0.5))
                    ex_tiles.append(ex)
                return ex_tiles

            def attn_av(ex_tiles, v_tiles, h, nkt, out_xh, sig_t=None):
                th, ro = h // 2, 64 * (h % 2)
                avps = ps_misc.tile([P, T], f32, tag="misc")
                for hcb in range(2):
                    cs = slice(hcb * HC, (hcb + 1) * HC)
                    for ktp in range(nkt // 2):
                        nc.tensor.matmul(
                            avps[:, cs],
                            v_tiles[ktp][:, :, h * VW:(h + 1) * VW],
                            ex_tiles[ktp][:, :, cs],
                            start=(ktp == 0), stop=(ktp == nkt // 2 - 1),
                            perf_mode=DR)
                rec = bcap.tile([64, T], bf16, tag="rec", bufs=2)
                nc.vector.reciprocal(rec[:, :], avps[64:128, :])
                dst = out_xh[ro:ro + 64, th, :]
                nc.vector.tensor_mul(dst, avps[0:64, :], rec[:, :])
                if sig_t is not None:
                    nc.gpsimd.tensor_mul(dst, dst, sig_t[th][ro:ro + 64, :])

            # ---------- SA attention interleaved with V / K2 / V2 -----------
            # per head: scores+exp (ACT-bound), PE fillers keep the tensor
            # engine busy; AV lags two heads so its exps are ready.
            sa_xh = acts.tile([P, CT, T], fp8, tag="sa_xh")
            wk2 = load_w("WK28")
            wv2 = load_w("WV28")
            k2_t = [acts.tile([P, T], bf16, tag=f"k2{i}", name=f"k2{i}")
                    for i in range(CT)]
            v2_tm = []
            for ktp in range(LKT // 2):
                vt = acts.tile([P, 2, H * VW], fp8, tag=f"v2tm{ktp}",
                               name=f"v2tm{ktp}")
                v2_tm.append(vt)
                nc.sync.dma_start(
                    vt[:, :, :],
                    din["vinit"].rearrange("p (j w) -> p j w", j=2))

            def evict_k2(oi, ps):
                nc.vector.tensor_scalar_mul(k2_t[oi][:, :], ps[:, :],
                                            1.0 / FP8S)

            sa_ex = {}
            for h in range(H):
                sa_ex[h] = attn_scores(q_t, k_t, h, NKT)
                # PE fillers
                if h < 6:
                    half, oc = h % 2, h // 2
                    xh_, br_ = (xh1, brows1) if half == 0 else (xh1o, brows1o)
                    for tj in range(4):
                        v_chunk(wv, xh_, br_, PROJ_V, v_tm, half, tj, oc)
                elif h == 6:
                    linear(wk2, xhc, browsc, PROJ_K2, evict_k2)
                else:
                    oc = h - 7
                    if oc < 3:
                        for tj in range(4):
                            v_chunk(wv2, xhc, browsc, PROJ_V2, v2_tm, 0, tj, oc)
                if h >= 2:
                    attn_av(sa_ex.pop(h - 2), v_tm, h - 2, NKT, sa_xh)
            for h in (H - 2, H - 1):
                attn_av(sa_ex.pop(h), v_tm, h, NKT, sa_xh)

            # ---------- proj_sa + gated residual ----------
            wp_ = load_w("WP8")

            def evict_res_gated(oi, ps):
                nc.vector.scalar_tensor_tensor(x_t[oi][:, :], ps[:, :],
                                               c_g64[:, oi:oi + 1],
                                               x_t[oi][:, :],
                                               ALU.mult, ALU.add)
            linear(wp_, sa_xh, brows1 if has_pbias else None,
                   PROJ_PSA if has_pbias else None, evict_res_gated)
            # x_t now holds x2

            # ---------- LN2 -> xh2; q2 / gate ----------
            bcA2, brows2 = ln_stats(x_t)
            xh2 = ln_xhat_dve(x_t, bcA2, "xh2")

            wqq = load_w("WQQ8")
            q2_t = [acts.tile([P, T], bf16, tag=f"q2{i}", name=f"q2{i}")
                    for i in range(CT)]

            def evict_q2(oi, ps):
                nc.vector.tensor_scalar_mul(q2_t[oi][:, :], ps[:, :], 1.0 / FP8S)
            linear(wqq, xh2, brows2, PROJ_QQ, evict_q2)

            wqg = load_w("WQG8")
            sig_t = [acts.tile([P, T], bf16, tag=f"sig{i}", name=f"sig{i}")
                     for i in range(CT)]

            def evict_sig(oi, ps):
                nc.scalar.activation(sig_t[oi][:, :], ps[:, :], AF.Sigmoid,
                                     scale=1.0 / FP8S)

            # ---------- CA attention (AV lags two heads) ----------
            # first two heads' scores go ahead of the gate sigmoids so the
            # ACT engine reaches the exp stream sooner
            ca_xh = acts.tile([P, CT, T], fp8, tag="ca_xh")
            ca_ex = {}
            for h in (0, 1):
                ca_ex[h] = attn_scores(q2_t, k2_t, h, LKT)
            linear(wqg, xh2, brows2, PROJ_QG, evict_sig)
            for h in range(2, H):
                ca_ex[h] = attn_scores(q2_t, k2_t, h, LKT)
                attn_av(ca_ex.pop(h - 2), v2_tm, h - 2, LKT, ca_xh,
                        sig_t=sig_t)
            for h in (H - 2, H - 1):
                attn_av(ca_ex.pop(h), v2_tm, h, LKT, ca_xh, sig_t=sig_t)

            # ---------- proj_ca + residual ----------
            wp2 = load_w("WP28")

            def evict_res(oi, ps):
                nc.vector.scalar_tensor_tensor(x_t[oi][:, :], ps[:, :],
                                               1.0 / FP8S, x_t[oi][:, :],
                                               ALU.mult, ALU.add)
                nc.sync.dma_start(x3_out[oi * P:(oi + 1) * P, :],
                                  x_t[oi][:, :])
            linear(wp2, ca_xh, brows1 if has_pbias else None,
                   PROJ_PCA if has_pbias else None, evict_res)
            # x_t now holds x3

            # ---------- LN3 -> xhat3 (f32r), logits, h2 export ----------
            bcA3, brows3 = ln_stats(x_t)
            bcB3 = ps_misc.tile([P, T], f32, tag="misc")
            nc.tensor.matmul(bcB3[:, :], ones_row[:, 0:P], brows3[0:1, :],
                             start=True, stop=True)
            bcs3b = bcap.tile([P, T], f32, tag="bcs")
            nc.vector.tensor_copy(bcs3b[:, :], bcB3[:, :])
            wr = vecp.tile([P, CT, 32], bf16, tag="wr")
            nc.sync.dma_start(wr[:, :, :],
                              din["WR"].rearrange("p (k e) -> p k e", k=CT))
            bcs3a = bcap.tile([P, T], f32, tag="bcs")
            nc.vector.tensor_copy(bcs3a[:, :], bcA3[:, :])
            xh3 = []
            for i in range(CT):
                t_ = acts.tile([P, T], bf16, tag=f"xh3_{i}")
                if i % 2 == 0:
                    nc.vector.tensor_mul(t_[:, :], x_t[i][:, :], bcA3[:, :])
                    nc.vector.tensor_add(t_[:, :], t_[:, :], bcs3b[:, :])
                else:
                    nc.gpsimd.tensor_mul(t_[:, :], x_t[i][:, :], bcs3a[:, :])
                    nc.gpsimd.tensor_add(t_[:, :], t_[:, :], bcs3b[:, :])
                xh3.append(t_)
            # token-major logits: out[tok%128, tj*32+e]
            lps = ps_misc.tile([P, T], f32, tag="misc")
            for tj in range(4):
                for i in range(CT):
                    nc.tensor.matmul(lps[:, tj * 32:(tj + 1) * 32],
                                     xh3[i][:, tj * P:(tj + 1) * P],
                                     wr[:, i, :],
                                     start=(i == 0), stop=(i == CT - 1))
            lg_sb = vecp.tile([P, 4 * 32], f32, tag="lg_sb")
            nc.vector.tensor_copy(lg_sb[:, :], lps[:, 0:128])
            nc.sync.dma_start(lg_out[:, :], lg_sb[:, :])

            h2pm = acts.tile([P, CT, T], fp8, tag="h2pm")
            for i in range(CT):
                if i % 2 == 0:
                    nc.scalar.activation(h2pm[:, i, :], xh3[i][:, :],
                                         AF.Identity,
                                         bias=c_shf[:, i:i + 1],
                                         scale=c_sc1f[:, i:i + 1])
                else:
                    nc.vector.tensor_scalar(h2pm[:, i, :], xh3[i][:, :],
                                            c_sc1f[:, i:i + 1],
                                            c_shf[:, i:i + 1],
                                            ALU.mult, ALU.add)
            h2o_ = h2_out.rearrange("p (k t) -> p k t", k=CT)
            nc.sync.dma_start(h2o_[:, 0:3, :], h2pm[:, 0:3, :])
            nc.sync.dma_start(h2o_[:, 3:CT, :], h2pm[:, 3:CT, :])

    nc.finalize()
    return nc



# --------------------------------------------------------------------------
# Pass B kernel builder: fp8 DoubleRow expert FFN over token bins.
#
# Per core: len(binsizes) bins; bin b holds tokens of ONE (virtual) expert,
# whose pre-scaled fp8 weights stream in per bin. Tokens are processed in
# 128-column chunks; each chunk's full h1 [F] lives across PSUM banks in two
# 12-plane halves so the gelu eviction runs as two wide ACT ops.
# --------------------------------------------------------------------------

FP8S = 64.0          # fp8 weight pre-scale (host multiplies W by this)
CHUNK = 128          # tokens per matmul chunk in pass B
PASS_B_CONFIGS = [(640, 640, 384), (640, 640, 512), (768, 768, 512),
                  (1024, 1024, 1024)]


def _build_pass_b(binsizes, with_bias):
    nc = bacc.Bacc("TRN2", target_bir_lowering=False, debug=False, num_devices=8)
    fp8 = mybir.dt.float8e4
    DR = mybir.MatmulPerfMode.DoubleRow
    NB = len(binsizes)
    BSMAX = max(binsizes)

    h2b = nc.dram_tensor("h2b", [NB, P, CT * BSMAX], fp8, kind="ExternalInput")
    w1b = nc.dram_tensor("w1b", [NB, P, CT * F], fp8, kind="ExternalInput")
    w2b = nc.dram_tensor("w2b", [NB, P, FT * C], fp8, kind="ExternalInput")
    if with_bias:
        b1c = nc.dram_tensor("b1c", [NB, P, FT], f32, kind="ExternalInput")
        b2c = nc.dram_tensor("b2c", [NB, P, CT], f32, kind="ExternalInput")
    y_out = nc.dram_tensor("y", [NB, P, CT * BSMAX], bf16, kind="ExternalOutput")

    with TileContext(nc) as tc:
        with tc.tile_pool(name="wp", bufs=2) as wp, \
             tc.tile_pool(name="hp", bufs=2) as hp, \
             tc.tile_pool(name="h1p", bufs=2) as h1p, \
             tc.tile_pool(name="yp", bufs=2) as yp, \
             tc.tile_pool(name="vec", bufs=2) as vecp, \
             tc.tile_pool(name="ps1", bufs=2, space="PSUM") as ps1p, \
             tc.tile_pool(name="psy", bufs=1, space="PSUM") as psyp:

            for b, BS in enumerate(binsizes):
                # h2 first (small), then w1 split per DoubleRow pair so the
                # first h1 matmuls start ~4us in; w2 streams under compute
                h2 = hp.tile([P, CT, BS], fp8, tag="h2")
                h2d_ = h2b[b].rearrange("p (k t) -> p k t", k=CT)
                nc.sync.dma_start(h2[:, 0:2, :], h2d_[:, 0:2, 0:BS])
                nc.sync.dma_start(h2[:, 2:CT, :], h2d_[:, 2:CT, 0:BS])
                w1 = wp.tile([P, CT, F], fp8, tag="w1")
                w1d = w1b[b].rearrange("p (k f) -> p k f", k=CT)
                for pi in range(CT // 2):
                    nc.sync.dma_start(w1[:, 2 * pi:2 * pi + 2, :],
                                      w1d[:, 2 * pi:2 * pi + 2, :])
                w2 = wp.tile([P, FT, C], fp8, tag="w2")
                w2d_ = w2b[b].rearrange("p (k f) -> p k f", k=FT)
                for wj in range(6):
                    nc.sync.dma_start(w2[:, 4 * wj:4 * (wj + 1), :],
                                      w2d_[:, 4 * wj:4 * (wj + 1), :])
                if with_bias:
                    b1 = vecp.tile([P, FT], f32, tag="b1")
                    nc.sync.dma_start(b1[:, :], b1c[b, :, :])
                    b2 = vecp.tile([P, CT], f32, tag="b2")
                    nc.sync.dma_start(b2[:, :], b2c[b, :, :])
                yt = yp.tile([P, CT, BS], bf16, tag="y")

                nch = BS // CHUNK
                for ch in range(nch):
                    t0 = ch * CHUNK
                    rhs_h2 = h2[:, :, t0:t0 + CHUNK]
                    h1 = h1p.tile([P, FT, CHUNK], fp8, tag="h1")
                    for half in range(2):
                        psh = ps1p.tile([P, FT // 2, CHUNK], f32, tag="psh")
                        for oj in range(FT // 2):
                            fo = (half * (FT // 2) + oj) * P
                            for pi in range(CT // 2):
                                nc.tensor.matmul(
                                    psh[:, oj, :],
                                    w1[:, 2 * pi:2 * pi + 2, fo:fo + P],
                                    rhs_h2[:, 2 * pi:2 * pi + 2, :],
                                    start=(pi == 0), stop=(pi == CT // 2 - 1),
                                    perf_mode=DR)
                        dst = h1[:, half * (FT // 2):(half + 1) * (FT // 2), :]
                        if with_bias:
                            for oj in range(FT // 2):
                                ojg = half * (FT // 2) + oj
                                nc.scalar.activation(
                                    dst[:, oj, :], psh[:, oj, :], AF.Gelu,
                                    bias=b1[:, ojg:ojg + 1], scale=1.0 / FP8S)
                        else:
                            nc.scalar.activation(dst[:, :, :], psh[:, :, :],
                                                 AF.Gelu, scale=1.0 / FP8S)
                    psy = psyp.tile([P, CT, CHUNK], f32, tag="psy")
                    for oi in range(CT):
                        for pj in range(FT // 2):
                            nc.tensor.matmul(
                                psy[:, oi, :],
                                w2[:, 2 * pj:2 * pj + 2, oi * P:(oi + 1) * P],
                                h1[:, 2 * pj:2 * pj + 2, :],
                                start=(pj == 0), stop=(pj == FT // 2 - 1),
                                perf_mode=DR)
                    ydst = yt[:, :, t0:t0 + CHUNK]
                    if with_bias:
                        for oi in range(CT):
                            nc.vector.tensor_scalar(
                                ydst[:, oi, :], psy[:, oi, :], 1.0 / FP8S,
                                b2[:, oi:oi + 1], ALU.mult, ALU.add)
                    else:
                        nc.vector.tensor_scalar_mul(ydst[:, :, :],
                                                    psy[:, :, :], 1.0 / FP8S)
                # split export: bulk leaves while the last chunk computes
                y_dst = y_out[b].rearrange("p (k t) -> p k t", k=CT)
                nc.sync.dma_start(y_dst[:, :, 0:BS - CHUNK],
                                  yt[:, :, 0:BS - CHUNK])
                nc.sync.dma_start(y_dst[:, :, BS - CHUNK:BS],
                                  yt[:, :, BS - CHUNK:BS])

    nc.finalize()
    return nc


def _get_nc(which):
    if which not in _CACHE:
        if which[0] == "a":
            _CACHE[which] = _build_pass_a(has_pbias=which[1])
        else:
            _, binsizes, with_bias = which
            _CACHE[which] = _build_pass_b(binsizes, with_bias)
    return _CACHE[which]


# --------------------------------------------------------------------------
# Host orchestration
# --------------------------------------------------------------------------

def _silu(x):
    return x / (1.0 + np.exp(-x))


def _softmax(x, axis=-1):
    x = x - x.max(axis=axis, keepdims=True)
    e = np.exp(x)
    return e / e.sum(axis=axis, keepdims=True)


def _ln_np(v, eps=EPS):
    m = v.mean(-1, keepdims=True)
    var = v.var(-1, keepdims=True)
    return (v - m) / np.sqrt(var + eps)


def _refine_logits(logits, amb, x, c, mod_vecs, tcond, W_qkv, Wqq, Wqg,
                   W_kv, Wp_sa, bp_sa, Wp_ca, bp_ca, W_router):
    """Recompute router logits exactly (fp32 host) for ambiguous tokens.

    The device pass runs matmuls in float32r (~11-bit mantissa), which is
    enough to route every token whose top-2 margin exceeds ~1e-4. For the
    handful of near-tie tokens, redo the whole block math for just those
    tokens in fp32 so the expert choice matches a full-precision reference.
    """
    f = np.float32
    sh_msa, sc_msa, g_msa, sh_mlp, sc_mlp, g_mlp, gamma, beta = mod_vecs
    scale = f(D) ** -0.5
    for b_ in np.unique(amb // N):
        tloc = amb[amb // N == b_] % N
        hb = _ln_np(x[b_]) * (1.0 + sc_msa[b_]) + sh_msa[b_]      # [N, C]
        k = (hb @ W_qkv[:, C:2 * C]).reshape(N, H, D)
        v = (hb @ W_qkv[:, 2 * C:]).reshape(N, H, D)
        q = (hb[tloc] @ W_qkv[:, :C]).reshape(-1, H, D)
        s = np.einsum('ahd,lhd->ahl', q * scale, k)
        s = np.exp(s - s.max(-1, keepdims=True))
        attn = s / s.sum(-1, keepdims=True)
        sa = np.einsum('ahl,lhd->ahd', attn, v).reshape(-1, C)
        sa = sa @ Wp_sa + bp_sa
        x2a = x[b_, tloc] + g_msa[b_] * sa
        cm = _ln_np(c[b_]) * gamma[b_] + beta[b_]
        k2 = (cm @ W_kv[:, :C]).reshape(L, H, D)
        v2 = (cm @ W_kv[:, C:]).reshape(L, H, D)
        hxa = _ln_np(x2a)
        q2 = (hxa @ Wqq).reshape(-1, H, D)
        gate = (hxa @ Wqg).reshape(-1, H, D)
        s2 = np.einsum('ahd,lhd->ahl', q2 * scale, k2)
        s2 = np.exp(s2 - s2.max(-1, keepdims=True))
        attn2 = s2 / s2.sum(-1, keepdims=True)
        ao = np.einsum('ahl,lhd->ahd', attn2, v2)
        ao = ao * (1.0 / (1.0 + np.exp(-gate)))
        ca = ao.reshape(-1, C) @ Wp_ca + bp_ca
        x3a = x2a + ca
        h2a = _ln_np(x3a) * (1.0 + sc_mlp[b_]) + sh_mlp[b_]
        logits[b_ * N + tloc] = h2a @ W_router + tcond[b_]
    return logits


def kernel(x, c, t, W_ada, b_ada, W_qkv, W_proj_sa, b_proj_sa, W_q, W_kv,
           W_proj_ca, b_proj_ca, W_cadaln, b_cadaln, W_router, W_tcond,
           W1, b1, W2, b2, Ws1, bs1, Ws2, bs2):
    f = np.float32
    x, c, t = np.asarray(x, f), np.asarray(c, f), np.asarray(t, f)

    # ---- host: tiny t-conditioned vectors (per batch) ----
    st = _silu(t)
    mod = st @ np.asarray(W_ada, f) + np.asarray(b_ada, f)          # [B, 6C]
    sh_msa, sc_msa, g_msa, sh_mlp, sc_mlp, g_mlp = np.split(mod, 6, axis=-1)
    gb = st @ np.asarray(W_cadaln, f) + np.asarray(b_cadaln, f)     # [B, 2C]
    gamma, beta = np.split(gb, 2, axis=-1)
    tcond = t @ np.asarray(W_tcond, f)                              # [B, E]

    # ---- pass A inputs ----
    fp8np = ml_dtypes.float8_e4m3fn
    W_qkv = np.asarray(W_qkv, f)
    Wq_sa = np.ascontiguousarray(W_qkv[:, :C])
    Wk_sa = np.ascontiguousarray(W_qkv[:, C:2 * C])
    Wv_sa = np.ascontiguousarray(W_qkv[:, 2 * C:])
    W_q = np.asarray(W_q, f).reshape(C, H, 2 * D)
    Wqq = np.ascontiguousarray(W_q[:, :, :D].reshape(C, C))
    Wqg = np.ascontiguousarray(W_q[:, :, D:].reshape(C, C))
    W_kv = np.asarray(W_kv, f)
    Wk_ca = np.ascontiguousarray(W_kv[:, :C])
    Wv_ca = np.ascontiguousarray(W_kv[:, C:])
    Wp_sa = np.asarray(W_proj_sa, f)
    Wp_ca = np.asarray(W_proj_ca, f)
    bp_sa = np.asarray(b_proj_sa, f)
    bp_ca = np.asarray(b_proj_ca, f)
    W_router = np.asarray(W_router, f)
    has_pbias = bool(bp_sa.any() or bp_ca.any())
    global LAST_A_KEY
    LAST_A_KEY = ("a", has_pbias)

    onesr = np.ones((1, T), f)
    onesc = np.ones((P, 1), f)
    vinit = np.zeros((P, 2 * H * VW), fp8np)
    for blk in range(2 * H):
        vinit[:, blk * VW + 64:(blk + 1) * VW] = 1.0

    ONE = np.ones(C, f)
    batch_maps = []
    for b_ in range(B):
        sc1m = 1.0 + sc_msa[b_]
        gam = gamma[b_]
        Wqs = Wq_sa * sc1m[:, None]
        Wks = Wk_sa * sc1m[:, None]
        Wvs = Wv_sa * sc1m[:, None]
        Wk2s = Wk_ca * gam[:, None]
        Wv2s = Wv_ca * gam[:, None]
        urows = np.zeros((2, NPROJ * C), f)
        for uix, (u_, w0_) in {
            PROJ_Q: (ONE @ Wqs, sh_msa[b_] @ Wq_sa),
            PROJ_K: (ONE @ Wks, sh_msa[b_] @ Wk_sa),
            PROJ_V: (ONE @ Wvs, sh_msa[b_] @ Wv_sa),
            PROJ_QQ: (ONE @ Wqq, np.zeros(C, f)),
            PROJ_QG: (ONE @ Wqg, np.zeros(C, f)),
            PROJ_K2: (ONE @ Wk2s, beta[b_] @ Wk_ca),
            PROJ_V2: (ONE @ Wv2s, beta[b_] @ Wv_ca),
            PROJ_PSA: (np.zeros(C, f), bp_sa),
            PROJ_PCA: (np.zeros(C, f), bp_ca),
        }.items():
            urows[0, uix * C:(uix + 1) * C] = FP8S * u_
            urows[1, uix * C:(uix + 1) * C] = FP8S * w0_
        cols = np.zeros((P, 3 * CT), f)
        for j, v in enumerate([1.0 + sc_mlp[b_], sh_mlp[b_],
                               g_msa[b_] / FP8S]):
            cols[:, j * CT:(j + 1) * CT] = v.reshape(CT, P).T
        WRs = np.zeros((C, 32), f)
        WRs[:, :E] = W_router * (1.0 + sc_mlp[b_])[:, None]
        batch_maps.append({
            "WQ8": prep_w(Wqs), "WK8": prep_w(Wks), "WV8": prep_w(Wvs),
            "WQQ8": prep_w(Wqq), "WQG8": prep_w(Wqg),
            "WK28": prep_w(Wk2s), "WV28": prep_w(Wv2s),
            "WP8": prep_w(Wp_sa), "WP28": prep_w(Wp_ca),
            "WR": np.ascontiguousarray(
                WRs.reshape(CT, P, 32).transpose(1, 0, 2)
                .reshape(P, CT * 32).astype(ml_dtypes.bfloat16)),
            "urows": urows, "cols": cols,
        })

    in_maps_a = []
    for core in range(8):
        b_, half = core // 2, core % 2
        sl = slice(half * T, (half + 1) * T)
        so = slice((1 - half) * T, (2 - half) * T)
        m = dict(batch_maps[b_])
        m.update({
            "xT": np.ascontiguousarray(x[b_, sl].T),
            "xoT": np.ascontiguousarray(x[b_, so].T).astype(ml_dtypes.bfloat16),
            "cT": np.ascontiguousarray(c[b_].T).astype(ml_dtypes.bfloat16),
            "onesr": onesr, "onesc": onesc, "vinit": vinit,
        })
        in_maps_a.append(m)

    nc_a = _get_nc(LAST_A_KEY)
    res_a = run_bass_kernel_spmd(nc_a, in_maps_a, core_ids=list(range(8)))

    x3 = np.empty((B, N, C), f)
    h2pm_full = np.empty((P, CT, B * N), fp8np)
    logits = np.empty((B * N, E), f)
    for core in range(8):
        b_, half = core // 2, core % 2
        sl = slice(half * T, (half + 1) * T)
        gsl = slice(b_ * N + half * T, b_ * N + half * T + T)
        x3[b_, sl] = res_a.results[core]["x3T"].T
        h2pm_full[:, :, gsl] = \
            res_a.results[core]["h2pm"].reshape(P, CT, T)
        lgarr = np.asarray(res_a.results[core]["lgT"], f)   # [P, 4*32]
        logits[gsl] = lgarr.reshape(P, 4, 32)[:, :, :E] \
            .transpose(1, 0, 2).reshape(T, E)

    # ---- host: router (fp32) + top-2 ----
    # device logits lack the per-batch sh_mlp @ W_router constant and tcond
    lg_const = sh_mlp @ W_router + tcond                       # [B, E]
    logits += np.repeat(lg_const, N, axis=0)
    probs = _softmax(logits, axis=-1)
    # near-tie tokens: fp8/f32r rounding on device could flip their top-2
    # choice vs a full-precision reference -- redo those on host in fp32
    ps_sorted = np.sort(probs, axis=-1)
    amb = np.nonzero(ps_sorted[:, -2] - ps_sorted[:, -3] < 1e-2)[0]
    if len(amb):
        mod_vecs = (sh_msa, sc_msa, g_msa, sh_mlp, sc_mlp, g_mlp, gamma, beta)
        logits = _refine_logits(logits, amb, x, c, mod_vecs, tcond, W_qkv,
                                Wqq, Wqg, W_kv, Wp_sa, bp_sa, Wp_ca, bp_ca,
                                W_router)
        probs[amb] = _softmax(logits[amb], axis=-1)
    order = np.argsort(-probs, axis=-1, kind="stable")
    topi = order[:, :TOPK]
    topv = np.take_along_axis(probs, topi, axis=-1)
    topv = topv / topv.sum(-1, keepdims=True)

    W1 = np.asarray(W1, f)
    W2 = np.asarray(W2, f)
    b1 = np.asarray(b1, f)
    b2 = np.asarray(b2, f)
    Ws1 = np.asarray(Ws1, f)
    Ws2 = np.asarray(Ws2, f)
    bs1 = np.asarray(bs1, f)
    bs2 = np.asarray(bs2, f)
    with_bias = bool(b1.any() or b2.any() or bs1.any() or bs2.any())

    # ---- bin packing: 8 cores x NB bins; each bin = tokens of one expert ----
    # expert e token list (order arbitrary), shared pseudo-expert = E
    tok_by_e = [np.nonzero(topi == e_)[0] for e_ in range(E)]
    wv_by_e = [topv[topi == e_] for e_ in range(E)]
    all_toks = np.arange(B * N)
    tok_by_e.append(all_toks)
    wv_by_e.append(np.ones(B * N, f))

    def pack(binsizes):
        """Assign expert pieces to the 8*len(binsizes) bins. Per-expert
        knapsack over bin-size counts with backtracking; shared expert
        fills whatever remains. Returns per-bin (expert, toks, wv) or None."""
        import itertools as _it
        nbins = 8 * len(binsizes)
        bin_sz = [binsizes[i % len(binsizes)] for i in range(nbins)]
        sizes = sorted(set(bin_sz), reverse=True)
        avail0 = tuple(sum(1 for s in bin_sz if s == sz) for sz in sizes)
        order = sorted(range(E), key=lambda e_: -len(tok_by_e[e_]))
        items = [len(tok_by_e[e_]) for e_ in order]
        shared_n = B * N

        def options(cnt, avail):
            opts = []
            maxn = [min(a, cnt // s + 2) for s, a in zip(sizes, avail)]
            for combo in _it.product(*[range(m + 1) for m in maxn]):
                tot = sum(n * s for n, s in zip(combo, sizes))
                if tot >= cnt and tot - cnt < sizes[-1]:
                    opts.append(combo)
            opts.sort(key=lambda c: sum(n * s for n, s in zip(c, sizes)))
            return opts[:40]

        def rec(idx, avail):
            if idx == len(items):
                if sum(a * s for a, s in zip(avail, sizes)) >= shared_n:
                    return []
                return None
            for combo in options(items[idx], avail):
                if all(n <= a for n, a in zip(combo, avail)):
                    sub = rec(idx + 1,
                              tuple(a - n for a, n in zip(avail, combo)))
                    if sub is not None:
                        return [combo] + sub
            return None

        combos = rec(0, avail0)
        if combos is None:
            return None
        # materialize: free bin ids per size
        free = {sz: [i for i in range(nbins) if bin_sz[i] == sz]
                for sz in sizes}
        assign = [None] * nbins
        for e_, combo in zip(order, combos):
            toks, wv = tok_by_e[e_], wv_by_e[e_]
            pos = 0
            for sz, n in zip(sizes, combo):
                for _ in range(n):
                    bid = free[sz].pop()
                    take = min(len(toks) - pos, sz)
                    if take > 0:
                        assign[bid] = (e_, toks[pos:pos + take],
                                       wv[pos:pos + take])
                        pos += take
        rem_bins = [i for sz in sizes for i in free[sz]]
        toks, wv = tok_by_e[E], wv_by_e[E]
        pos = 0
        for bid in rem_bins:
            take = min(len(toks) - pos, bin_sz[bid])
            if take > 0:
                assign[bid] = (E, toks[pos:pos + take], wv[pos:pos + take])
                pos += take
        if pos < shared_n:
            return None
        return assign

    assign = None
    for cfg in PASS_B_CONFIGS:
        assign = pack(list(cfg))
        if assign is not None:
            binsizes = list(cfg)
            break
    assert assign is not None, "no pass-B config fits this routing"
    global LAST_B_KEY
    LAST_B_KEY = ("b", tuple(binsizes), with_bias)

    NB = len(binsizes)
    BSMAX = max(binsizes)

    w1_pre = [prep_w(W1[e_]) for e_ in range(E)] + [prep_w(Ws1)]
    w2_pre = [prep_w(W2[e_]) for e_ in range(E)] + [prep_w(Ws2)]
    b1_all = np.concatenate([b1, bs1[None]], 0)   # [E+1, F]
    b2_all = np.concatenate([b2, bs2[None]], 0)   # [E+1, C]

    # h2 tokens already fp8 partition-major from pass A: h2pm_full [P,CT,BN]
    h2pm = h2pm_full

    in_maps_b = []
    for core in range(8):
        h2bin = np.zeros((NB, P, CT * BSMAX), fp8np)
        w1bin = np.empty((NB, P, CT * F), fp8np)
        w2bin = np.empty((NB, P, FT * C), fp8np)
        m = {"h2b": h2bin, "w1b": w1bin, "w2b": w2bin}
        if with_bias:
            m["b1c"] = np.zeros((NB, P, FT), f)
            m["b2c"] = np.zeros((NB, P, CT), f)
        for s in range(NB):
            a = assign[core * NB + s]
            BS = binsizes[s]
            if a is None:
                w1bin[s] = 0
                w2bin[s] = 0
                continue
            e_, toks, _wv = a
            w1bin[s] = w1_pre[e_]
            w2bin[s] = w2_pre[e_]
            h2bin[s].reshape(P, CT, BSMAX)[:, :, :len(toks)] = h2pm[:, :, toks]
            if with_bias:
                m["b1c"][s] = b1_all[e_].reshape(FT, P).T
                m["b2c"][s] = b2_all[e_].reshape(CT, P).T
        in_maps_b.append(m)

    nc_b = _get_nc(LAST_B_KEY)
    res_b = run_bass_kernel_spmd(nc_b, in_maps_b, core_ids=list(range(8)))

    # ---- host: weighted scatter-add combine + final residual ----
    accum = np.zeros((B * N, C), f)
    for core in range(8):
        y = np.asarray(res_b.results[core]["y"], f)  # [NB, P, CT*BSMAX]
        for s in range(NB):
            a = assign[core * NB + s]
            if a is None:
                continue
            e_, toks, wv = a
            # y[p, k, t] = out feature k*P+p of token t
            yv = y[s].reshape(P, CT, BSMAX)[:, :, :len(toks)]
            accum[toks] += wv[:, None] * yv.transpose(2, 1, 0).reshape(-1, C)

    out = x3 + g_mlp[:, None, :] * accum.reshape(B, N, C)
    return out.astype(np.float32)



# revision 20
# speedup vs baseline: 1.0951x; 1.0951x over previous
"""Trainium2 Bass kernel for nn_DiTXMoEBlock (DiT block: adaLN + self-attn +
gated cross-attn + top-2-of-8 MoE FFN + shared expert).

Strategy (8 NeuronCores, full inputs in / full output out):
- Pass A (data-parallel, 512 query tokens per core = half a batch): the
  two attention blocks, ending at x3 (the MoE input residual). All heavy
  matmuls run as fp8e4m3 DoubleRow (2 k-planes per PE pass). The adaLN
  scales are folded into the weights on the host (per batch); the
  per-token LN shift enters via a rank-2 correction matmul [u;w0]^T[b;1]
  accumulated into each psum. V tiles are token-major with 64
  ones-columns per head so the AV matmul also produces the softmax
  row-sum broadcast across 64 psum rows; a single DVE divide finishes
  each head. Self-attention score/exp streams (ACT-bound) are
  interleaved with the V/K2/V2 projections and lag-2 AV matmuls to keep
  the tensor engine busy. The CA gate uses tanh (same ACT table set as
  exp, avoiding 1.3us table swaps): sigmoid(z) = (1+tanh(z/2))/2, with
  the 2x folded into W_proj_ca.
- Host: LN3 + adaLN modulate + router + top-2 in fp32 from the exported
  x3; near-tie tokens (margin < 1e-2) are re-derived in full fp32 so
  routing matches the reference exactly. h2 is quantized to fp8
  partition-major for pass B.
- Pass B (bin-packed expert FFN, fp8 DoubleRow): each bin holds one
  expert's tokens (shared expert fills the slack). Tokens run in
  512-wide matmul chunks (the PE sequencer is off the critical path);
  gelu evicts h1 in wide 3-plane ACT ops; y leaves as bf16.
- Host: weighted scatter-add combine + final residual.
"""

import numpy as np

try:
    import concourse.bacc as bacc
except ImportError:  # fall back to the repo checkout location
    import sys
    sys.path.insert(0, "/opt/trn_rl_repo")
    import concourse.bacc as bacc

import ml_dtypes
import concourse.bass as bass
import concourse.mybir as mybir
from concourse.bass_utils import run_bass_kernel_spmd
from concourse.tile import TileContext

AF = mybir.ActivationFunctionType
ALU = mybir.AluOpType
f32 = mybir.dt.float32
f32r = mybir.dt.float32r
bf16 = mybir.dt.bfloat16

B, N, L, C, H, E, TOPK, F = 4, 1024, 512, 768, 12, 8, 2, 3072
D = C // H          # 64
P = 128             # partitions
T = 512             # tokens per core in pass A (half a batch)
CT = C // P         # 6 C-tiles
FT = F // P         # 24 F-tiles
NKT = N // P        # 8 key tiles (self-attention, full batch seq)
LKT = L // P        # 4 key tiles (cross-attention context)
EPS = 1e-5
NSLOT = 4           # expert-chunk slots per core in pass B
CAP = 512           # tokens per chunk slot
VW = 128            # per-head width in v_tm layout (64 v cols + 64 ones cols
                    # whose AV product replicates the softmax row-sum)

_CACHE = {}
LAST_A_KEY = ("a", False)   # pass-A variant used on the last kernel() call
LAST_B_KEY = None           # pass-B variant used on the last kernel() call


def prep_w(Wmat):
    """[K, M] fp32 -> [P, (K//P)*M] fp8, scaled by FP8S, partition-major:
    w[p, k, m] = FP8S * W[k*P+p, m]."""
    Kd, Md = Wmat.shape
    return np.ascontiguousarray(
        (Wmat * FP8S).reshape(Kd // P, P, Md).transpose(1, 0, 2)
        .reshape(P, (Kd // P) * Md).astype(ml_dtypes.float8_e4m3fn))


# --------------------------------------------------------------------------
# Pass A kernel builder (fp8 DoubleRow linears, LN scale folded into weights)
# --------------------------------------------------------------------------

# rank-2 row indices in the urows input (u = col-sums of scaled W, w0 = W^T sh)
NPROJ = 9           # q, k, v, qq, qg, k2, v2, psa, pca
(PROJ_Q, PROJ_K, PROJ_V, PROJ_QQ, PROJ_QG, PROJ_K2, PROJ_V2,
 PROJ_PSA, PROJ_PCA) = range(NPROJ)

def _build_pass_a(has_pbias=False):
    fp8 = mybir.dt.float8e4
    DR = mybir.MatmulPerfMode.DoubleRow
    nc = bacc.Bacc("TRN2", target_bir_lowering=False, debug=False, num_devices=8)

    din = {}
    for nm, shape, dt_ in [
        ("xT", [C, T], f32r), ("xoT", [C, T], bf16), ("cT", [C, T], bf16),
        ("WQ8", [P, CT * C], fp8), ("WK8", [P, CT * C], fp8),
        ("WV8", [P, CT * C], fp8), ("WQQ8", [P, CT * C], fp8),
        ("WQG8", [P, CT * C], fp8), ("WK28", [P, CT * C], fp8),
        ("WV28", [P, CT * C], fp8), ("WP8", [P, CT * C], fp8),
        ("WP28", [P, CT * C], fp8),
        ("urows", [2, NPROJ * C], f32r),
        ("cols", [P, CT], f32),
        ("onesr", [1, T], f32r), ("onesc", [P, 1], f32r),
        ("vinit", [P, 2 * H * VW], fp8),
    ]:
        din[nm] = nc.dram_tensor(nm, shape, dt_, kind="ExternalInput")
    x3_out = nc.dram_tensor("x3T", [C, T], f32r, kind="ExternalOutput")

    with TileContext(nc) as tc, \
         nc.allow_low_precision("fp8/f32r rounding of matmul operands is intended"):
        with tc.tile_pool(name="acts", bufs=1) as acts, \
             tc.tile_pool(name="wpool", bufs=3) as wpool, \
             tc.tile_pool(name="vec", bufs=1) as vecp, \
             tc.tile_pool(name="sq", bufs=4) as sqp, \
             tc.tile_pool(name="exps", bufs=12) as expp, \
             tc.tile_pool(name="bca", bufs=3) as bcap, \
             tc.tile_pool(name="ps_lin", bufs=2, space="PSUM") as ps_lin, \
             tc.tile_pool(name="ps_sc", bufs=2, space="PSUM") as ps_sc, \
             tc.tile_pool(name="ps_misc", bufs=2, space="PSUM") as ps_misc:

            # ---------- constants ----------
            ones_col = vecp.tile([P, 1], f32r, tag="ones_col")
            nc.sync.dma_start(ones_col[:, :], din["onesc"][:, :])
            ones_col_b = vecp.tile([P, 1], bf16, tag="ones_col_b")
            nc.vector.memset(ones_col_b[:, :], 1.0)

            ones_row = vecp.tile([1, T], f32r, tag="ones_row")
            nc.sync.dma_start(ones_row[:, :], din["onesr"][:, :])
            eps_t = vecp.tile([1, 1], f32, tag="eps")
            nc.vector.memset(eps_t[:, :], EPS)
            cols = vecp.tile([P, CT], f32, tag="cols")
            nc.sync.dma_start(cols[:, :], din["cols"][:, :])
            c_g64 = cols[:, 0:CT]                   # g_msa / FP8S
            urows = vecp.tile([2, NPROJ * C], f32r, tag="urows")
            nc.sync.dma_start(urows[:, :], din["urows"][:, :])

            # ---------- activations ----------
            def load_fm(name, tag, dt_=f32r):
                ts = []
                for i in range(CT):
                    t_ = acts.tile([P, T], dt_, tag=f"{tag}{i}")
                    nc.sync.dma_start(t_[:, :], din[name][i * P:(i + 1) * P, :])
                    ts.append(t_)
                return ts

            x_t = load_fm("xT", "x")

            def load_w(name):
                w = wpool.tile([P, CT, C], fp8, tag="wblk")
                nc.sync.dma_start(
                    w[:, :, :], din[name].rearrange("p (k c) -> p k c", k=CT))
                return w

            # ---------- LayerNorm stats ----------
            def ln_stats(src_tiles, sq_engine="pool", oc_=None):
                """Returns bcA [P,T] f32 PSUM tile (rstd broadcast) and
                brows [2,T] f32r = (b = -mean*rstd ; ones)."""
                oc_ = ones_col if oc_ is None else oc_
                st_x = ps_misc.tile([P, T], f32, tag="misc")
                for i in range(CT):
                    nc.tensor.matmul(st_x[0:1, :], oc_[:, :],
                                     src_tiles[i][:, :],
                                     start=(i == 0), stop=(i == CT - 1))
                st_xx = ps_misc.tile([P, T], f32, tag="misc")
                for i in range(CT):
                    sq = sqp.tile([P, T], bf16, tag="sq")
                    # alternate engines: halves the serial square-chain latency
                    if (i % 2 == 0) == (sq_engine == "pool"):
                        nc.gpsimd.tensor_mul(sq[:, :], src_tiles[i][:, :],
                                             src_tiles[i][:, :])
                    else:
                        nc.vector.tensor_mul(sq[:, :], src_tiles[i][:, :],
                                             src_tiles[i][:, :])
                    nc.tensor.matmul(st_xx[0:1, :], ones_col_b[:, :], sq[:, :],
                                     start=(i == 0), stop=(i == CT - 1))
                m2 = vecp.tile([1, T], f32, tag="m2", bufs=2)
                nc.scalar.square(m2[:, :], st_x[0:1, :])
                varp = vecp.tile([1, T], f32, tag="varp", bufs=2)
                nc.vector.scalar_tensor_tensor(varp[:, :], m2[:, :], -1.0 / C,
                                               st_xx[0:1, :], ALU.mult, ALU.add)
                sd = vecp.tile([1, T], f32, tag="sd", bufs=2)
                nc.scalar.activation(sd[:, :], varp[:, :], AF.Sqrt,
                                     bias=eps_t[:, 0:1], scale=1.0 / C)
                a = vecp.tile([1, T], f32r, tag="a", bufs=2)
                nc.vector.reciprocal(a[:, :], sd[:, :])
                brows = vecp.tile([2, T], f32r, tag="brows", bufs=4)
                nc.sync.dma_start(brows[1:2, :], din["onesr"][:, :])
                nc.vector.scalar_tensor_tensor(brows[0:1, :], st_x[0:1, :],
                                               -1.0 / C, a[:, :],
                                               ALU.mult, ALU.mult)
                bcA = ps_misc.tile([P, T], f32, tag="misc")
                nc.tensor.matmul(bcA[:, :], ones_row[:, 0:P], a[:, :],
                                 start=True, stop=True)
                return bcA, brows

            def ln_xhat_dve(src_tiles, bcA, tag):
                bcs = bcap.tile([P, T], f32, tag="bcs")
                nc.vector.tensor_copy(bcs[:, :], bcA[:, :])
                xh = acts.tile([P, CT, T], fp8, tag=tag)
                for i in range(CT):
                    if i % 2 == 0:
                        nc.vector.tensor_mul(xh[:, i, :], src_tiles[i][:, :],
                                             bcA[:, :])
                    else:
                        nc.gpsimd.tensor_mul(xh[:, i, :], src_tiles[i][:, :],
                                             bcs[:, :])
                return xh

            def ln_xhat_pool(src_tiles, bcA, tag):
                # even planes on DVE (psum bcA), odd planes on Pool (sbuf
                # copy) -- halves the serial latency of the apply chain
                bcs = bcap.tile([P, T], f32, tag="bcs")
                nc.vector.tensor_copy(bcs[:, :], bcA[:, :])
                xh = acts.tile([P, CT, T], fp8, tag=tag)
                for i in range(CT):
                    if i % 2 == 0:
                        nc.vector.tensor_mul(xh[:, i, :], src_tiles[i][:, :],
                                             bcA[:, :])
                    else:
                        nc.gpsimd.tensor_mul(xh[:, i, :], src_tiles[i][:, :],
                                             bcs[:, :])
                return xh

            # ---------- fp8 DoubleRow linear (feature-major out) ----------
            HC = T // 2    # 256-token matmul chunks

            def linear_oi(w, xh, brows, uix, evict, oi):
                    ps = ps_lin.tile([P, T], f32, tag="lin")
                    for hcb in range(2):
                        cs = slice(hcb * HC, (hcb + 1) * HC)
                        for pi in range(CT // 2):
                            nc.tensor.matmul(
                                ps[:, cs], w[:, 2 * pi:2 * pi + 2,
                                             oi * P:(oi + 1) * P],
                                xh[:, 2 * pi:2 * pi + 2, cs],
                                start=(pi == 0),
                                stop=(pi == CT // 2 - 1 and uix is None),
                                perf_mode=DR)
                        if uix is not None:
                            nc.tensor.matmul(
                                ps[:, cs],
                                urows[0:2,
                                      uix * C + oi * P:uix * C + (oi + 1) * P],
                                brows[0:2, cs], start=False, stop=True)
                    evict(oi, ps)

            def linear(w, xh, brows, uix, evict):
                for oi in range(CT):
                    linear_oi(w, xh, brows, uix, evict, oi)

            # ---------- LN1 + Q/K (xo/c stats stream in behind) ----------
            wq = load_w("WQ8")
            wk = load_w("WK8")
            bcA1, brows1 = ln_stats(x_t, sq_engine="dve")
            xh1 = ln_xhat_dve(x_t, bcA1, "xh1")

            q_t = [acts.tile([P, T], bf16, tag=f"q{i}", name=f"q{i}")
                   for i in range(CT)]

            def evict_q(oi, ps):
                nc.vector.tensor_scalar_mul(q_t[oi][:, :], ps[:, :], 1.0 / FP8S)
            linear(wq, xh1, brows1, PROJ_Q, evict_q)

            k_t = [acts.tile([P, N], bf16, tag=f"k{i}", name=f"k{i}")
                   for i in range(CT)]

            def evict_k0(oi, ps):
                nc.vector.tensor_scalar_mul(k_t[oi][:, 0:T], ps[:, :],
                                            1.0 / FP8S)
            linear(wk, xh1, brows1, PROJ_K, evict_k0)

            xo_t = load_fm("xoT", "xo", bf16)
            bcA1o, brows1o = ln_stats(xo_t, oc_=ones_col_b)
            xh1o = ln_xhat_pool(xo_t, bcA1o, "xh1o")

            def evict_k1(oi, ps):
                nc.vector.tensor_scalar_mul(k_t[oi][:, T:N], ps[:, :],
                                            1.0 / FP8S)
            linear(wk, xh1o, brows1o, PROJ_K, evict_k1)

            c_t = load_fm("cT", "c", bf16)
            bcAc, browsc = ln_stats(c_t, oc_=ones_col_b)
            xhc = ln_xhat_pool(c_t, bcAc, "xhc")

            # ---------- V tiles (token-major interleaved fp8 pairs) ---------
            wv = load_w("WV8")
            v_tm = []
            for ktp in range(NKT // 2):
                vt = acts.tile([P, 2, H * VW], fp8, tag=f"vtm{ktp}",
                               name=f"vtm{ktp}")
                v_tm.append(vt)
                nc.sync.dma_start(
                    vt[:, :, :],
                    din["vinit"].rearrange("p (j w) -> p j w", j=2))

            def v_chunk(wv_, xh_, br_, uix, v_tiles, half, tj, oc):
                """One [128tok, 256feat] chunk of the V projection."""
                kt = half * 4 + tj
                ktp, par = kt // 2, kt % 2
                ps = ps_lin.tile([P, T], f32, tag="lin")
                for pi in range(CT // 2):
                    nc.tensor.matmul(
                        ps[:, 0:256],
                        xh_[:, 2 * pi:2 * pi + 2, tj * P:(tj + 1) * P],
                        wv_[:, 2 * pi:2 * pi + 2, oc * 256:(oc + 1) * 256],
                        start=(pi == 0), stop=False, perf_mode=DR)
                nc.tensor.matmul(
                    ps[:, 0:256], br_[0:2, tj * P:(tj + 1) * P],
                    urows[0:2, uix * C + oc * 256:uix * C + (oc + 1) * 256],
                    start=False, stop=True)
                vb = v_tiles[ktp][:, par, :]
                dst = bass.AP(vb.tensor, vb.offset + oc * 4 * VW,
                              [list(vb.ap[0]), [VW, 4], [1, 64]])
                nc.vector.tensor_scalar_mul(
                    dst, ps[:, 0:256].rearrange("p (h d) -> p h d", h=4),
                    1.0 / FP8S)

            # ---------- attention helpers (VW=128: ones cols -> rowsum) -----
            def attn_scores(q_tiles, k_tiles, h, nkt):
                th, ro = h // 2, 64 * (h % 2)
                ex_tiles = []
                for ktp in range(nkt // 2):
                    sps = ps_sc.tile([P, 2, T], f32, tag="score")
                    for par in range(2):
                        kt = 2 * ktp + par
                        nc.tensor.matmul(
                            sps[:, par, :],
                            k_tiles[th][ro:ro + 64, kt * P:(kt + 1) * P],
                            q_tiles[th][ro:ro + 64, :],
                            start=True, stop=True)
                    ex = expp.tile([P, 2, T], fp8, tag="exp")
                    nc.scalar.activation(ex[:, :, :], sps[:, :, :], AF.Exp,
                                         scale=float(D ** -0.5))
                    ex_tiles.append(ex)
                return ex_tiles

            def attn_av(ex_tiles, v_tiles, h, nkt, out_xh, gate_t=None):
                th, ro = h // 2, 64 * (h % 2)
                avps = ps_misc.tile([P, T], f32, tag="misc")
                for hcb in range(2):
                    cs = slice(hcb * HC, (hcb + 1) * HC)
                    for ktp in range(nkt // 2):
                        nc.tensor.matmul(
                            avps[:, cs],
                            v_tiles[ktp][:, :, h * VW:(h + 1) * VW],
                            ex_tiles[ktp][:, :, cs],
                            start=(ktp == 0), stop=(ktp == nkt // 2 - 1),
                            perf_mode=DR)
                rec = bcap.tile([64, T], bf16, tag="rec", bufs=2)
                nc.vector.reciprocal(rec[:, :], avps[64:128, :])
                dst = out_xh[ro:ro + 64, th, :]
                nc.vector.tensor_mul(dst, avps[0:64, :], rec[:, :])
                if gate_t is not None:
                    # gate_t holds tanh(z/2); dst *= (1 + tanh) = 2*sigmoid(z)
                    # (the extra 2x is folded into W_proj_ca on the host)
                    nc.vector.scalar_tensor_tensor(dst,
                                                   gate_t[th][ro:ro + 64, :],
                                                   1.0, dst,
                                                   ALU.add, ALU.mult)

            # ---------- SA attention interleaved with V / K2 / V2 -----------
            # per head: scores+exp (ACT-bound), PE fillers keep the tensor
            # engine busy; AV lags two heads so its exps are ready.
            sa_xh = acts.tile([P, CT, T], fp8, tag="sa_xh")
            wk2 = load_w("WK28")
            wv2 = load_w("WV28")
            k2_t = [acts.tile([P, T], bf16, tag=f"k2{i}", name=f"k2{i}")
                    for i in range(CT)]
            v2_tm = []
            for ktp in range(LKT // 2):
                vt = acts.tile([P, 2, H * VW], fp8, tag=f"v2tm{ktp}",
                               name=f"v2tm{ktp}")
                v2_tm.append(vt)
                nc.sync.dma_start(
                    vt[:, :, :],
                    din["vinit"].rearrange("p (j w) -> p j w", j=2))

            def evict_k2(oi, ps):
                nc.vector.tensor_scalar_mul(k2_t[oi][:, :], ps[:, :],
                                            1.0 / FP8S)

            sa_ex = {}
            for h in range(H):
                sa_ex[h] = attn_scores(q_t, k_t, h, NKT)
                # PE fillers
                if h < 6:
                    half, oc = h % 2, h // 2
                    xh_, br_ = (xh1, brows1) if half == 0 else (xh1o, brows1o)
                    for tj in range(4):
                        v_chunk(wv, xh_, br_, PROJ_V, v_tm, half, tj, oc)
                elif h == 6:
                    linear(wk2, xhc, browsc, PROJ_K2, evict_k2)
                else:
                    oc = h - 7
                    if oc < 3:
                        for tj in range(4):
                            v_chunk(wv2, xhc, browsc, PROJ_V2, v2_tm, 0, tj, oc)
                if h >= 2:
                    attn_av(sa_ex.pop(h - 2), v_tm, h - 2, NKT, sa_xh)
            for h in (H - 2, H - 1):
                attn_av(sa_ex.pop(h), v_tm, h, NKT, sa_xh)

            # ---------- proj_sa + gated residual ----------
            wp_ = load_w("WP8")

            def evict_res_gated(oi, ps):
                nc.vector.scalar_tensor_tensor(x_t[oi][:, :], ps[:, :],
                                               c_g64[:, oi:oi + 1],
                                               x_t[oi][:, :],
                                               ALU.mult, ALU.add)
            linear(wp_, sa_xh, brows1 if has_pbias else None,
                   PROJ_PSA if has_pbias else None, evict_res_gated)
            # x_t now holds x2

            # ---------- LN2 -> xh2; q2 / gate ----------
            bcA2, brows2 = ln_stats(x_t)
            xh2 = ln_xhat_dve(x_t, bcA2, "xh2")

            wqq = load_w("WQQ8")
            q2_t = [acts.tile([P, T], bf16, tag=f"q2{i}", name=f"q2{i}")
                    for i in range(CT)]

            def evict_q2(oi, ps):
                nc.vector.tensor_scalar_mul(q2_t[oi][:, :], ps[:, :], 1.0 / FP8S)
            linear(wqq, xh2, brows2, PROJ_QQ, evict_q2)

            wqg = load_w("WQG8")
            sig_t = [acts.tile([P, T], bf16, tag=f"sig{i}", name=f"sig{i}")
                     for i in range(CT)]

            def evict_sig(oi, ps):
                # tanh(z/2) lives in the exp act-table set (sigmoid does
                # not); sigmoid = (1 + tanh(z/2)) / 2 is finished in attn_av
                nc.scalar.activation(sig_t[oi][:, :], ps[:, :], AF.Tanh,
                                     scale=0.5 / FP8S)

            # ---------- CA attention (AV lags two heads) ----------
            # first two heads' scores go ahead of the gate sigmoids so the
            # ACT engine reaches the exp stream sooner
            ca_xh = acts.tile([P, CT, T], fp8, tag="ca_xh")
            ca_ex = {}
            for h in (0, 1):
                ca_ex[h] = attn_scores(q2_t, k2_t, h, LKT)
            linear(wqg, xh2, brows2, PROJ_QG, evict_sig)
            for h in range(2, H):
                ca_ex[h] = attn_scores(q2_t, k2_t, h, LKT)
                attn_av(ca_ex.pop(h - 2), v2_tm, h - 2, LKT, ca_xh,
                        gate_t=sig_t)
            for h in (H - 2, H - 1):
                attn_av(ca_ex.pop(h), v2_tm, h, LKT, ca_xh, gate_t=sig_t)

            # ---------- proj_ca + residual ----------
            wp2 = load_w("WP28")

            def evict_res(oi, ps):
                nc.vector.scalar_tensor_tensor(x_t[oi][:, :], ps[:, :],
                                               1.0 / FP8S, x_t[oi][:, :],
                                               ALU.mult, ALU.add)
                nc.sync.dma_start(x3_out[oi * P:(oi + 1) * P, :],
                                  x_t[oi][:, :])
            linear(wp2, ca_xh, brows1 if has_pbias else None,
                   PROJ_PCA if has_pbias else None, evict_res)
            # x_t now holds x3; LN3 / router / h2 all run on the host

    nc.finalize()
    return nc



# --------------------------------------------------------------------------
# Pass B kernel builder: fp8 DoubleRow expert FFN over token bins.
#
# Per core: len(binsizes) bins; bin b holds tokens of ONE (virtual) expert,
# whose pre-scaled fp8 weights stream in per bin. Tokens run in 512-wide
# matmul chunks (ap=512 keeps the PE sequencer off the critical path, vs
# ap=128 where Ldweights dispatch saturates PE.SEQ); h1 is produced in
# 3-plane PSUM groups (3 banks x 2 bufs) evicted by wide gelu ACT ops, and
# y accumulates per-oi in single-bank PSUM tiles evicted on DVE.
# --------------------------------------------------------------------------

FP8S = 64.0          # fp8 weight pre-scale (host multiplies W by this)
CHUNK = 512          # tokens per matmul chunk in pass B
GG = 3               # h1 psum group: 3 oj planes (3 banks; bufs=2 -> 6)
PASS_B_CONFIGS = [(640, 640, 384), (640, 640, 512), (768, 768, 512),
                  (1024, 1024, 1024)]


def _build_pass_b(binsizes, with_bias):
    nc = bacc.Bacc("TRN2", target_bir_lowering=False, debug=False, num_devices=8)
    fp8 = mybir.dt.float8e4
    DR = mybir.MatmulPerfMode.DoubleRow
    NB = len(binsizes)
    BSMAX = max(binsizes)

    h2b = nc.dram_tensor("h2b", [NB, P, CT * BSMAX], fp8, kind="ExternalInput")
    w1b = nc.dram_tensor("w1b", [NB, P, CT * F], fp8, kind="ExternalInput")
    w2b = nc.dram_tensor("w2b", [NB, P, FT * C], fp8, kind="ExternalInput")
    if with_bias:
        b1c = nc.dram_tensor("b1c", [NB, P, FT], f32, kind="ExternalInput")
        b2c = nc.dram_tensor("b2c", [NB, P, CT], f32, kind="ExternalInput")
    y_out = nc.dram_tensor("y", [NB, P, CT * BSMAX], bf16, kind="ExternalOutput")

    def chunks_of(BS):
        out = []
        t0 = 0
        while t0 < BS:
            out.append((t0, min(CHUNK, BS - t0)))
            t0 += CHUNK
        return out

    with TileContext(nc) as tc:
        with tc.tile_pool(name="wp", bufs=2) as wp, \
             tc.tile_pool(name="hp", bufs=2) as hp, \
             tc.tile_pool(name="h1p", bufs=2) as h1p, \
             tc.tile_pool(name="yp", bufs=2) as yp, \
             tc.tile_pool(name="vec", bufs=2) as vecp, \
             tc.tile_pool(name="ps1", bufs=2, space="PSUM") as ps1p, \
             tc.tile_pool(name="psy", bufs=2, space="PSUM") as psyp:

            for b, BS in enumerate(binsizes):
                # h2 first (small), then w1 split per DoubleRow pair so the
                # first h1 matmuls start early; w2 streams under compute
                h2 = hp.tile([P, CT, BS], fp8, tag="h2")
                h2d_ = h2b[b].rearrange("p (k t) -> p k t", k=CT)
                nc.sync.dma_start(h2[:, 0:2, :], h2d_[:, 0:2, 0:BS])
                nc.sync.dma_start(h2[:, 2:CT, :], h2d_[:, 2:CT, 0:BS])
                w1 = wp.tile([P, CT, F], fp8, tag="w1")
                w1d = w1b[b].rearrange("p (k f) -> p k f", k=CT)
                for pi in range(CT // 2):
                    nc.sync.dma_start(w1[:, 2 * pi:2 * pi + 2, :],
                                      w1d[:, 2 * pi:2 * pi + 2, :])
                w2 = wp.tile([P, FT, C], fp8, tag="w2")
                w2d_ = w2b[b].rearrange("p (k f) -> p k f", k=FT)
                for wj in range(6):
                    nc.sync.dma_start(w2[:, 4 * wj:4 * (wj + 1), :],
                                      w2d_[:, 4 * wj:4 * (wj + 1), :])
                if with_bias:
                    b1 = vecp.tile([P, FT], f32, tag="b1")
                    nc.sync.dma_start(b1[:, :], b1c[b, :, :])
                    b2 = vecp.tile([P, CT], f32, tag="b2")
                    nc.sync.dma_start(b2[:, :], b2c[b, :, :])
                yt = yp.tile([P, CT, BS], bf16, tag="y")

                for t0, CH in chunks_of(BS):
                    rhs_h2 = h2[:, :, t0:t0 + CH]
                    h1 = h1p.tile([P, FT, CHUNK], fp8, tag="h1")
                    for g in range(FT // GG):
                        psh = ps1p.tile([P, GG, CHUNK], f32, tag="psh")
                        for oj in range(GG):
                            fo = (g * GG + oj) * P
                            for pi in range(CT // 2):
                                nc.tensor.matmul(
                                    psh[:, oj, 0:CH],
                                    w1[:, 2 * pi:2 * pi + 2, fo:fo + P],
                                    rhs_h2[:, 2 * pi:2 * pi + 2, :],
                                    start=(pi == 0), stop=(pi == CT // 2 - 1),
                                    perf_mode=DR)
                        dst = h1[:, g * GG:(g + 1) * GG, 0:CH]
                        if with_bias:
                            for oj in range(GG):
                                ojg = g * GG + oj
                                nc.scalar.activation(
                                    dst[:, oj, :], psh[:, oj, 0:CH], AF.Gelu,
                                    bias=b1[:, ojg:ojg + 1], scale=1.0 / FP8S)
                        else:
                            nc.scalar.activation(dst[:, :, :],
                                                 psh[:, :, 0:CH],
                                                 AF.Gelu, scale=1.0 / FP8S)
                    for oi in range(CT):
                        psy = psyp.tile([P, CHUNK], f32, tag="psy")
                        for pj in range(FT // 2):
                            nc.tensor.matmul(
                                psy[:, 0:CH],
                                w2[:, 2 * pj:2 * pj + 2, oi * P:(oi + 1) * P],
                                h1[:, 2 * pj:2 * pj + 2, 0:CH],
                                start=(pj == 0), stop=(pj == FT // 2 - 1),
                                perf_mode=DR)
                        ydst = yt[:, oi, t0:t0 + CH]
                        if with_bias:
                            nc.vector.tensor_scalar(
                                ydst, psy[:, 0:CH], 1.0 / FP8S,
                                b2[:, oi:oi + 1], ALU.mult, ALU.add)
                        else:
                            nc.vector.tensor_scalar_mul(ydst, psy[:, 0:CH],
                                                        1.0 / FP8S)
                # split export: bulk leaves while the last chunk computes
                y_dst = y_out[b].rearrange("p (k t) -> p k t", k=CT)
                if BS > CHUNK:
                    nc.sync.dma_start(y_dst[:, :, 0:BS - CHUNK],
                                      yt[:, :, 0:BS - CHUNK])
                    nc.sync.dma_start(y_dst[:, :, BS - CHUNK:BS],
                                      yt[:, :, BS - CHUNK:BS])
                else:
                    nc.sync.dma_start(y_dst[:, :, 0:BS], yt[:, :, 0:BS])

    nc.finalize()
    return nc


def _get_nc(which):
    if which not in _CACHE:
        if which[0] == "a":
            _CACHE[which] = _build_pass_a(has_pbias=which[1])
        else:
            _, binsizes, with_bias = which
            _CACHE[which] = _build_pass_b(binsizes, with_bias)
    return _CACHE[which]


# --------------------------------------------------------------------------
# Host orchestration
# --------------------------------------------------------------------------

def _silu(x):
    return x / (1.0 + np.exp(-x))


def _softmax(x, axis=-1):
    x = x - x.max(axis=axis, keepdims=True)
    e = np.exp(x)
    return e / e.sum(axis=axis, keepdims=True)


def _ln_np(v, eps=EPS):
    m = v.mean(-1, keepdims=True)
    var = v.var(-1, keepdims=True)
    return (v - m) / np.sqrt(var + eps)


def _refine_logits(logits, amb, x, c, mod_vecs, tcond, W_qkv, Wqq, Wqg,
                   W_kv, Wp_sa, bp_sa, Wp_ca, bp_ca, W_router):
    """Recompute router logits exactly (fp32 host) for ambiguous tokens.

    The device pass runs matmuls in float32r (~11-bit mantissa), which is
    enough to route every token whose top-2 margin exceeds ~1e-4. For the
    handful of near-tie tokens, redo the whole block math for just those
    tokens in fp32 so the expert choice matches a full-precision reference.
    """
    f = np.float32
    sh_msa, sc_msa, g_msa, sh_mlp, sc_mlp, g_mlp, gamma, beta = mod_vecs
    scale = f(D) ** -0.5
    for b_ in np.unique(amb // N):
        tloc = amb[amb // N == b_] % N
        hb = _ln_np(x[b_]) * (1.0 + sc_msa[b_]) + sh_msa[b_]      # [N, C]
        k = (hb @ W_qkv[:, C:2 * C]).reshape(N, H, D)
        v = (hb @ W_qkv[:, 2 * C:]).reshape(N, H, D)
        q = (hb[tloc] @ W_qkv[:, :C]).reshape(-1, H, D)
        s = np.einsum('ahd,lhd->ahl', q * scale, k)
        s = np.exp(s - s.max(-1, keepdims=True))
        attn = s / s.sum(-1, keepdims=True)
        sa = np.einsum('ahl,lhd->ahd', attn, v).reshape(-1, C)
        sa = sa @ Wp_sa + bp_sa
        x2a = x[b_, tloc] + g_msa[b_] * sa
        cm = _ln_np(c[b_]) * gamma[b_] + beta[b_]
        k2 = (cm @ W_kv[:, :C]).reshape(L, H, D)
        v2 = (cm @ W_kv[:, C:]).reshape(L, H, D)
        hxa = _ln_np(x2a)
        q2 = (hxa @ Wqq).reshape(-1, H, D)
        gate = (hxa @ Wqg).reshape(-1, H, D)
        s2 = np.einsum('ahd,lhd->ahl', q2 * scale, k2)
        s2 = np.exp(s2 - s2.max(-1, keepdims=True))
        attn2 = s2 / s2.sum(-1, keepdims=True)
        ao = np.einsum('ahl,lhd->ahd', attn2, v2)
        ao = ao * (1.0 / (1.0 + np.exp(-gate)))
        ca = ao.reshape(-1, C) @ Wp_ca + bp_ca
        x3a = x2a + ca
        h2a = _ln_np(x3a) * (1.0 + sc_mlp[b_]) + sh_mlp[b_]
        logits[b_ * N + tloc] = h2a @ W_router + tcond[b_]
    return logits


def kernel(x, c, t, W_ada, b_ada, W_qkv, W_proj_sa, b_proj_sa, W_q, W_kv,
           W_proj_ca, b_proj_ca, W_cadaln, b_cadaln, W_router, W_tcond,
           W1, b1, W2, b2, Ws1, bs1, Ws2, bs2):
    f = np.float32
    x, c, t = np.asarray(x, f), np.asarray(c, f), np.asarray(t, f)

    # ---- host: tiny t-conditioned vectors (per batch) ----
    st = _silu(t)
    mod = st @ np.asarray(W_ada, f) + np.asarray(b_ada, f)          # [B, 6C]
    sh_msa, sc_msa, g_msa, sh_mlp, sc_mlp, g_mlp = np.split(mod, 6, axis=-1)
    gb = st @ np.asarray(W_cadaln, f) + np.asarray(b_cadaln, f)     # [B, 2C]
    gamma, beta = np.split(gb, 2, axis=-1)
    tcond = t @ np.asarray(W_tcond, f)                              # [B, E]

    # ---- pass A inputs ----
    fp8np = ml_dtypes.float8_e4m3fn
    W_qkv = np.asarray(W_qkv, f)
    Wq_sa = np.ascontiguousarray(W_qkv[:, :C])
    Wk_sa = np.ascontiguousarray(W_qkv[:, C:2 * C])
    Wv_sa = np.ascontiguousarray(W_qkv[:, 2 * C:])
    W_q = np.asarray(W_q, f).reshape(C, H, 2 * D)
    Wqq = np.ascontiguousarray(W_q[:, :, :D].reshape(C, C))
    Wqg = np.ascontiguousarray(W_q[:, :, D:].reshape(C, C))
    W_kv = np.asarray(W_kv, f)
    Wk_ca = np.ascontiguousarray(W_kv[:, :C])
    Wv_ca = np.ascontiguousarray(W_kv[:, C:])
    Wp_sa = np.asarray(W_proj_sa, f)
    Wp_ca = np.asarray(W_proj_ca, f)
    bp_sa = np.asarray(b_proj_sa, f)
    bp_ca = np.asarray(b_proj_ca, f)
    W_router = np.asarray(W_router, f)
    has_pbias = bool(bp_sa.any() or bp_ca.any())
    global LAST_A_KEY
    LAST_A_KEY = ("a", has_pbias)

    onesr = np.ones((1, T), f)
    onesc = np.ones((P, 1), f)
    vinit = np.zeros((P, 2 * H * VW), fp8np)
    for blk in range(2 * H):
        vinit[:, blk * VW + 64:(blk + 1) * VW] = 1.0

    ONE = np.ones(C, f)
    batch_maps = []
    for b_ in range(B):
        sc1m = 1.0 + sc_msa[b_]
        gam = gamma[b_]
        Wqs = Wq_sa * sc1m[:, None]
        Wks = Wk_sa * sc1m[:, None]
        Wvs = Wv_sa * sc1m[:, None]
        Wk2s = Wk_ca * gam[:, None]
        Wv2s = Wv_ca * gam[:, None]
        urows = np.zeros((2, NPROJ * C), f)
        for uix, (u_, w0_) in {
            PROJ_Q: (ONE @ Wqs, sh_msa[b_] @ Wq_sa),
            PROJ_K: (ONE @ Wks, sh_msa[b_] @ Wk_sa),
            PROJ_V: (ONE @ Wvs, sh_msa[b_] @ Wv_sa),
            PROJ_QQ: (ONE @ Wqq, np.zeros(C, f)),
            PROJ_QG: (ONE @ Wqg, np.zeros(C, f)),
            PROJ_K2: (ONE @ Wk2s, beta[b_] @ Wk_ca),
            PROJ_V2: (ONE @ Wv2s, beta[b_] @ Wv_ca),
            PROJ_PSA: (np.zeros(C, f), bp_sa),
            PROJ_PCA: (np.zeros(C, f), bp_ca),
        }.items():
            urows[0, uix * C:(uix + 1) * C] = FP8S * u_
            urows[1, uix * C:(uix + 1) * C] = FP8S * w0_
        cols = np.ascontiguousarray((g_msa[b_] / FP8S).reshape(CT, P).T)
        batch_maps.append({
            "WQ8": prep_w(Wqs), "WK8": prep_w(Wks), "WV8": prep_w(Wvs),
            "WQQ8": prep_w(Wqq), "WQG8": prep_w(Wqg),
            "WK28": prep_w(Wk2s), "WV28": prep_w(Wv2s),
            # 0.5x: the CA gate is computed as (1 + tanh(z/2)) = 2*sigmoid(z)
            "WP8": prep_w(Wp_sa), "WP28": prep_w(0.5 * Wp_ca),
            "urows": urows, "cols": cols,
        })

    in_maps_a = []
    for core in range(8):
        b_, half = core // 2, core % 2
        sl = slice(half * T, (half + 1) * T)
        so = slice((1 - half) * T, (2 - half) * T)
        m = dict(batch_maps[b_])
        m.update({
            "xT": np.ascontiguousarray(x[b_, sl].T),
            "xoT": np.ascontiguousarray(x[b_, so].T).astype(ml_dtypes.bfloat16),
            "cT": np.ascontiguousarray(c[b_].T).astype(ml_dtypes.bfloat16),
            "onesr": onesr, "onesc": onesc, "vinit": vinit,
        })
        in_maps_a.append(m)

    nc_a = _get_nc(LAST_A_KEY)
    res_a = run_bass_kernel_spmd(nc_a, in_maps_a, core_ids=list(range(8)))

    x3 = np.empty((B, N, C), f)
    for core in range(8):
        b_, half = core // 2, core % 2
        sl = slice(half * T, (half + 1) * T)
        x3[b_, sl] = res_a.results[core]["x3T"].T

    # ---- host: LN3 + adaLN modulate + router (fp32) + top-2 ----
    xhat3 = _ln_np(x3)                                         # [B, N, C]
    h2_full = xhat3 * (1.0 + sc_mlp[:, None, :]) + sh_mlp[:, None, :]
    logits = (h2_full @ W_router + tcond[:, None, :]).reshape(B * N, E)
    h2pm_full = np.ascontiguousarray(
        h2_full.reshape(B * N, CT, P).transpose(2, 1, 0)).astype(fp8np)
    probs = _softmax(logits, axis=-1)
    # near-tie tokens: fp8/f32r rounding on device could flip their top-2
    # choice vs a full-precision reference -- redo those on host in fp32
    ps_sorted = np.sort(probs, axis=-1)
    amb = np.nonzero(ps_sorted[:, -2] - ps_sorted[:, -3] < 1e-2)[0]
    if len(amb):
        mod_vecs = (sh_msa, sc_msa, g_msa, sh_mlp, sc_mlp, g_mlp, gamma, beta)
        logits = _refine_logits(logits, amb, x, c, mod_vecs, tcond, W_qkv,
                                Wqq, Wqg, W_kv, Wp_sa, bp_sa, Wp_ca, bp_ca,
                                W_router)
        probs[amb] = _softmax(logits[amb], axis=-1)
    order = np.argsort(-probs, axis=-1, kind="stable")
    topi = order[:, :TOPK]
    topv = np.take_along_axis(probs, topi, axis=-1)
    topv = topv / topv.sum(-1, keepdims=True)

    W1 = np.asarray(W1, f)
    W2 = np.asarray(W2, f)
    b1 = np.asarray(b1, f)
    b2 = np.asarray(b2, f)
    Ws1 = np.asarray(Ws1, f)
    Ws2 = np.asarray(Ws2, f)
    bs1 = np.asarray(bs1, f)
    bs2 = np.asarray(bs2, f)
    with_bias = bool(b1.any() or b2.any() or bs1.any() or bs2.any())

    # ---- bin packing: 8 cores x NB bins; each bin = tokens of one expert ----
    # expert e token list (order arbitrary), shared pseudo-expert = E
    tok_by_e = [np.nonzero(topi == e_)[0] for e_ in range(E)]
    wv_by_e = [topv[topi == e_] for e_ in range(E)]
    all_toks = np.arange(B * N)
    tok_by_e.append(all_toks)
    wv_by_e.append(np.ones(B * N, f))

    def pack(binsizes):
        """Assign expert pieces to the 8*len(binsizes) bins. Per-expert
        knapsack over bin-size counts with backtracking; shared expert
        fills whatever remains. Returns per-bin (expert, toks, wv) or None."""
        import itertools as _it
        nbins = 8 * len(binsizes)
        bin_sz = [binsizes[i % len(binsizes)] for i in range(nbins)]
        sizes = sorted(set(bin_sz), reverse=True)
        avail0 = tuple(sum(1 for s in bin_sz if s == sz) for sz in sizes)
        order = sorted(range(E), key=lambda e_: -len(tok_by_e[e_]))
        items = [len(tok_by_e[e_]) for e_ in order]
        shared_n = B * N

        def options(cnt, avail):
            opts = []
            maxn = [min(a, cnt // s + 2) for s, a in zip(sizes, avail)]
            for combo in _it.product(*[range(m + 1) for m in maxn]):
                tot = sum(n * s for n, s in zip(combo, sizes))
                if tot >= cnt and tot - cnt < sizes[-1]:
                    opts.append(combo)
            opts.sort(key=lambda c: sum(n * s for n, s in zip(c, sizes)))
            return opts[:40]

        def rec(idx, avail):
            if idx == len(items):
                if sum(a * s for a, s in zip(avail, sizes)) >= shared_n:
                    return []
                return None
            for combo in options(items[idx], avail):
                if all(n <= a for n, a in zip(combo, avail)):
                    sub = rec(idx + 1,
                              tuple(a - n for a, n in zip(avail, combo)))
                    if sub is not None:
                        return [combo] + sub
            return None

        combos = rec(0, avail0)
        if combos is None:
            return None
        # materialize: free bin ids per size
        free = {sz: [i for i in range(nbins) if bin_sz[i] == sz]
                for sz in sizes}
        assign = [None] * nbins
        for e_, combo in zip(order, combos):
            toks, wv = tok_by_e[e_], wv_by_e[e_]
            pos = 0
            for sz, n in zip(sizes, combo):
                for _ in range(n):
                    bid = free[sz].pop()
                    take = min(len(toks) - pos, sz)
                    if take > 0:
                        assign[bid] = (e_, toks[pos:pos + take],
                                       wv[pos:pos + take])
                        pos += take
        rem_bins = [i for sz in sizes for i in free[sz]]
        toks, wv = tok_by_e[E], wv_by_e[E]
        pos = 0
        for bid in rem_bins:
            take = min(len(toks) - pos, bin_sz[bid])
            if take > 0:
                assign[bid] = (E, toks[pos:pos + take], wv[pos:pos + take])
                pos += take
        if pos < shared_n:
            return None
        return assign

    assign = None
    for cfg in PASS_B_CONFIGS:
        assign = pack(list(cfg))
        if assign is not None:
            binsizes = list(cfg)
            break
    assert assign is not None, "no pass-B config fits this routing"
    global LAST_B_KEY
    LAST_B_KEY = ("b", tuple(binsizes), with_bias)

    NB = len(binsizes)
    BSMAX = max(binsizes)

    w1_pre = [prep_w(W1[e_]) for e_ in range(E)] + [prep_w(Ws1)]
    w2_pre = [prep_w(W2[e_]) for e_ in range(E)] + [prep_w(Ws2)]
    b1_all = np.concatenate([b1, bs1[None]], 0)   # [E+1, F]
    b2_all = np.concatenate([b2, bs2[None]], 0)   # [E+1, C]

    # h2 tokens in fp8 partition-major from the host LN3: h2pm_full [P,CT,BN]
    h2pm = h2pm_full

    in_maps_b = []
    for core in range(8):
        h2bin = np.zeros((NB, P, CT * BSMAX), fp8np)
        w1bin = np.empty((NB, P, CT * F), fp8np)
        w2bin = np.empty((NB, P, FT * C), fp8np)
        m = {"h2b": h2bin, "w1b": w1bin, "w2b": w2bin}
        if with_bias:
            m["b1c"] = np.zeros((NB, P, FT), f)
            m["b2c"] = np.zeros((NB, P, CT), f)
        for s in range(NB):
            a = assign[core * NB + s]
            BS = binsizes[s]
            if a is None:
                w1bin[s] = 0
                w2bin[s] = 0
                continue
            e_, toks, _wv = a
            w1bin[s] = w1_pre[e_]
            w2bin[s] = w2_pre[e_]
            h2bin[s].reshape(P, CT, BSMAX)[:, :, :len(toks)] = h2pm[:, :, toks]
            if with_bias:
                m["b1c"][s] = b1_all[e_].reshape(FT, P).T
                m["b2c"][s] = b2_all[e_].reshape(CT, P).T
        in_maps_b.append(m)

    nc_b = _get_nc(LAST_B_KEY)
    res_b = run_bass_kernel_spmd(nc_b, in_maps_b, core_ids=list(range(8)))

    # ---- host: weighted scatter-add combine + final residual ----
    accum = np.zeros((B * N, C), f)
    for core in range(8):
        y = np.asarray(res_b.results[core]["y"], f)  # [NB, P, CT*BSMAX]
        for s in range(NB):
            a = assign[core * NB + s]
            if a is None:
                continue
            e_, toks, wv = a
            # y[p, k, t] = out feature k*P+p of token t
            yv = y[s].reshape(P, CT, BSMAX)[:, :, :len(toks)]
            accum[toks] += wv[:, None] * yv.transpose(2, 1, 0).reshape(-1, C)

    out = x3 + g_mlp[:, None, :] * accum.reshape(B, N, C)
    return out.astype(np.float32)



# revision 33
# speedup vs baseline: 1.1009x; 1.0054x over previous
"""Trainium2 Bass kernel for nn_DiTXMoEBlock (DiT block: adaLN + self-attn +
gated cross-attn + top-2-of-8 MoE FFN + shared expert).

Strategy (8 NeuronCores, full inputs in / full output out):
- Pass A (data-parallel, 512 query tokens per core = half a batch): the
  two attention blocks, ending at x3 (the MoE input residual). All heavy
  matmuls run as fp8e4m3 DoubleRow (2 k-planes per PE pass). The adaLN
  scales are folded into the weights on the host (per batch); the
  per-token LN shift enters via a rank-2 correction matmul [u;w0]^T[b;1]
  accumulated into each psum. V tiles are token-major with 64
  ones-columns per head so the AV matmul also produces the softmax
  row-sum broadcast across 64 psum rows; a single DVE divide finishes
  each head. Self-attention score/exp streams (ACT-bound) are
  interleaved with the V/K2/V2 projections and lag-2 AV matmuls to keep
  the tensor engine busy. The CA gate uses tanh (same ACT table set as
  exp, avoiding 1.3us table swaps): sigmoid(z) = (1+tanh(z/2))/2, with
  the 2x folded into W_proj_ca.
- Host: LN3 + adaLN modulate + router + top-2 in fp32 from the exported
  x3; near-tie tokens (margin < 1e-2) are re-derived in full fp32 so
  routing matches the reference exactly. h2 is quantized to fp8
  partition-major for pass B.
- Pass B (bin-packed expert FFN, fp8 DoubleRow): each bin holds one
  expert's tokens (shared expert fills the slack). Tokens run in
  512-wide matmul chunks (the PE sequencer is off the critical path);
  gelu evicts h1 in wide 3-plane ACT ops; y leaves as bf16.
- Host: weighted scatter-add combine + final residual.
"""

import numpy as np

try:
    import concourse.bacc as bacc
except ImportError:  # fall back to the repo checkout location
    import sys
    sys.path.insert(0, "/opt/trn_rl_repo")
    import concourse.bacc as bacc

import ml_dtypes
import concourse.bass as bass
import concourse.mybir as mybir
from concourse.bass_utils import run_bass_kernel_spmd
from concourse.tile import TileContext

AF = mybir.ActivationFunctionType
ALU = mybir.AluOpType
f32 = mybir.dt.float32
f32r = mybir.dt.float32r
bf16 = mybir.dt.bfloat16

B, N, L, C, H, E, TOPK, F = 4, 1024, 512, 768, 12, 8, 2, 3072
D = C // H          # 64
P = 128             # partitions
T = 512             # tokens per core in pass A (half a batch)
CT = C // P         # 6 C-tiles
FT = F // P         # 24 F-tiles
NKT = N // P        # 8 key tiles (self-attention, full batch seq)
LKT = L // P        # 4 key tiles (cross-attention context)
EPS = 1e-5
NSLOT = 4           # expert-chunk slots per core in pass B
CAP = 512           # tokens per chunk slot
VW = 128            # per-head width in v_tm layout (64 v cols + 64 ones cols
                    # whose AV product replicates the softmax row-sum)

_CACHE = {}
LAST_A_KEY = ("a", False)   # pass-A variant used on the last kernel() call
LAST_B_KEY = None           # pass-B variant used on the last kernel() call


def prep_w(Wmat):
    """[K, M] fp32 -> [P, (K//P)*M] fp8, scaled by FP8S, partition-major:
    w[p, k, m] = FP8S * W[k*P+p, m]."""
    Kd, Md = Wmat.shape
    return np.ascontiguousarray(
        (Wmat * FP8S).reshape(Kd // P, P, Md).transpose(1, 0, 2)
        .reshape(P, (Kd // P) * Md).astype(ml_dtypes.float8_e4m3fn))


# --------------------------------------------------------------------------
# Pass A kernel builder (fp8 DoubleRow linears, LN scale folded into weights)
# --------------------------------------------------------------------------

# rank-2 row indices in the urows input (u = col-sums of scaled W, w0 = W^T sh)
NPROJ = 9           # q, k, v, qq, qg, k2, v2, psa, pca
(PROJ_Q, PROJ_K, PROJ_V, PROJ_QQ, PROJ_QG, PROJ_K2, PROJ_V2,
 PROJ_PSA, PROJ_PCA) = range(NPROJ)

def _build_pass_a(has_pbias=False):
    fp8 = mybir.dt.float8e4
    DR = mybir.MatmulPerfMode.DoubleRow
    nc = bacc.Bacc("TRN2", target_bir_lowering=False, debug=False, num_devices=8)

    din = {}
    for nm, shape, dt_ in [
        ("xT", [C, T], f32r), ("xoT", [C, T], bf16), ("cT", [C, T], bf16),
        ("WQ8", [P, CT * C], fp8), ("WK8", [P, CT * C], fp8),
        ("WV8", [P, CT * C], fp8), ("WQQ8", [P, CT * C], fp8),
        ("WQG8", [P, CT * C], fp8), ("WK28", [P, CT * C], fp8),
        ("WV28", [P, CT * C], fp8), ("WP8", [P, CT * C], fp8),
        ("WP28", [P, CT * C], fp8),
        ("urows", [1, 2 * NPROJ * C], fp8),
        ("cols", [P, CT], f32),
        ("onesr", [1, T], f32r), ("onesc", [P, 1], f32r),
        ("vinit", [P, 2 * H * VW], fp8),
    ]:
        din[nm] = nc.dram_tensor(nm, shape, dt_, kind="ExternalInput")
    x3_out = nc.dram_tensor("x3T", [C, T], f32r, kind="ExternalOutput")

    with TileContext(nc) as tc, \
         nc.allow_low_precision("fp8/f32r rounding of matmul operands is intended"):
        with tc.tile_pool(name="acts", bufs=1) as acts, \
             tc.tile_pool(name="wpool", bufs=3) as wpool, \
             tc.tile_pool(name="vec", bufs=1) as vecp, \
             tc.tile_pool(name="sq", bufs=4) as sqp, \
             tc.tile_pool(name="exps", bufs=12) as expp, \
             tc.tile_pool(name="bca", bufs=3) as bcap, \
             tc.tile_pool(name="ps_lin", bufs=2, space="PSUM") as ps_lin, \
             tc.tile_pool(name="ps_sc", bufs=2, space="PSUM") as ps_sc, \
             tc.tile_pool(name="ps_misc", bufs=2, space="PSUM") as ps_misc:

            # ---------- constants ----------
            ones_col = vecp.tile([P, 1], f32r, tag="ones_col")
            nc.sync.dma_start(ones_col[:, :], din["onesc"][:, :])
            ones_col_b = vecp.tile([P, 1], bf16, tag="ones_col_b")
            nc.vector.memset(ones_col_b[:, :], 1.0)


            ones_row = vecp.tile([1, T], f32r, tag="ones_row")
            nc.sync.dma_start(ones_row[:, :], din["onesr"][:, :])
            eps_t = vecp.tile([1, 1], f32, tag="eps")
            nc.vector.memset(eps_t[:, :], EPS)
            cols = vecp.tile([P, CT], f32, tag="cols")
            nc.sync.dma_start(cols[:, :], din["cols"][:, :])
            c_g64 = cols[:, 0:CT]                   # g_msa / FP8S
            urows = vecp.tile([1, 2, NPROJ * C], fp8, tag="urows")
            nc.sync.dma_start(
                urows[:, :, :],
                din["urows"].rearrange("a (j m) -> a j m", j=2))

            # ---------- activations ----------
            def load_fm(name, tag, dt_=f32r):
                ts = []
                for i in range(CT):
                    t_ = acts.tile([P, T], dt_, tag=f"{tag}{i}")
                    nc.sync.dma_start(t_[:, :], din[name][i * P:(i + 1) * P, :])
                    ts.append(t_)
                return ts

            x_t = load_fm("xT", "x")

            def load_w(name):
                w = wpool.tile([P, CT, C], fp8, tag="wblk")
                nc.sync.dma_start(
                    w[:, :, :], din[name].rearrange("p (k c) -> p k c", k=CT))
                return w

            # ---------- LayerNorm stats ----------
            def ln_stats(src_tiles, sq_engine="pool", oc_=None):
                """Returns bcA [P,T] f32 PSUM tile (rstd broadcast) and
                brows [1,2,T] fp8 = (b = -mean*rstd ; ones), DoubleRow-ready
                for the rank-2 shift-correction matmuls."""
                oc_ = ones_col if oc_ is None else oc_
                st_x = ps_misc.tile([P, T], f32, tag="misc")
                for i in range(CT):
                    nc.tensor.matmul(st_x[0:1, :], oc_[:, :],
                                     src_tiles[i][:, :],
                                     start=(i == 0), stop=(i == CT - 1))
                st_xx = ps_misc.tile([P, T], f32, tag="misc")
                for i in range(CT):
                    sq = sqp.tile([P, T], bf16, tag="sq")
                    # alternate engines: halves the serial square-chain latency
                    if (i % 2 == 0) == (sq_engine == "pool"):
                        nc.gpsimd.tensor_mul(sq[:, :], src_tiles[i][:, :],
                                             src_tiles[i][:, :])
                    else:
                        nc.vector.tensor_mul(sq[:, :], src_tiles[i][:, :],
                                             src_tiles[i][:, :])
                    nc.tensor.matmul(st_xx[0:1, :], ones_col_b[:, :], sq[:, :],
                                     start=(i == 0), stop=(i == CT - 1))
                m2 = vecp.tile([1, T], f32, tag="m2", bufs=2)
                nc.scalar.square(m2[:, :], st_x[0:1, :])
                varp = vecp.tile([1, T], f32, tag="varp", bufs=2)
                nc.vector.scalar_tensor_tensor(varp[:, :], m2[:, :], -1.0 / C,
                                               st_xx[0:1, :], ALU.mult, ALU.add)
                sd = vecp.tile([1, T], f32, tag="sd", bufs=2)
                nc.scalar.activation(sd[:, :], varp[:, :], AF.Sqrt,
                                     bias=eps_t[:, 0:1], scale=1.0 / C)
                a = vecp.tile([1, T], f32r, tag="a", bufs=2)
                nc.vector.reciprocal(a[:, :], sd[:, :])
                brows = vecp.tile([1, 2, T], fp8, tag="brows", bufs=4)
                nc.vector.memset(brows[0:1, 1, :], 1.0)
                nc.vector.scalar_tensor_tensor(brows[0:1, 0, :], st_x[0:1, :],
                                               -1.0 / C, a[:, :],
                                               ALU.mult, ALU.mult)
                bcA = ps_misc.tile([P, T], f32, tag="misc")
                nc.tensor.matmul(bcA[:, :], ones_row[:, 0:P], a[:, :],
                                 start=True, stop=True)
                return bcA, brows

            def ln_xhat_dve(src_tiles, bcA, tag):
                bcs = bcap.tile([P, T], f32, tag="bcs")
                nc.vector.tensor_copy(bcs[:, :], bcA[:, :])
                xh = acts.tile([P, CT, T], fp8, tag=tag)
                for i in range(CT):
                    if i % 2 == 0:
                        nc.vector.tensor_mul(xh[:, i, :], src_tiles[i][:, :],
                                             bcA[:, :])
                    else:
                        nc.gpsimd.tensor_mul(xh[:, i, :], src_tiles[i][:, :],
                                             bcs[:, :])
                return xh

            def ln_xhat_pool(src_tiles, bcA, tag):
                # even planes on DVE (psum bcA), odd planes on Pool (sbuf
                # copy) -- halves the serial latency of the apply chain
                bcs = bcap.tile([P, T], f32, tag="bcs")
                nc.vector.tensor_copy(bcs[:, :], bcA[:, :])
                xh = acts.tile([P, CT, T], fp8, tag=tag)
                for i in range(CT):
                    if i % 2 == 0:
                        nc.vector.tensor_mul(xh[:, i, :], src_tiles[i][:, :],
                                             bcA[:, :])
                    else:
                        nc.gpsimd.tensor_mul(xh[:, i, :], src_tiles[i][:, :],
                                             bcs[:, :])
                return xh

            # ---------- fp8 DoubleRow linear (feature-major out) ----------
            HC = T // 2    # 256-token matmul chunks

            def linear_oi(w, xh, brows, uix, evict, oi):
                    ps = ps_lin.tile([P, T], f32, tag="lin")
                    for hcb in range(2):
                        cs = slice(hcb * HC, (hcb + 1) * HC)
                        for pi in range(CT // 2):
                            nc.tensor.matmul(
                                ps[:, cs], w[:, 2 * pi:2 * pi + 2,
                                             oi * P:(oi + 1) * P],
                                xh[:, 2 * pi:2 * pi + 2, cs],
                                start=(pi == 0),
                                stop=(pi == CT // 2 - 1 and uix is None),
                                perf_mode=DR)
                        if uix is not None:
                            nc.tensor.matmul(
                                ps[:, cs],
                                urows[0:1, :,
                                      uix * C + oi * P:uix * C + (oi + 1) * P],
                                brows[0:1, :, cs], start=False, stop=True,
                                perf_mode=DR)
                    evict(oi, ps)

            def linear(w, xh, brows, uix, evict):
                for oi in range(CT):
                    linear_oi(w, xh, brows, uix, evict, oi)

            # ---------- LN1 + Q/K (xo/c stats stream in behind) ----------
            wq = load_w("WQ8")
            wk = load_w("WK8")
            bcA1, brows1 = ln_stats(x_t, sq_engine="dve")
            xh1 = ln_xhat_dve(x_t, bcA1, "xh1")

            q_t = [acts.tile([P, T], bf16, tag=f"q{i}", name=f"q{i}")
                   for i in range(CT)]

            def evict_q(oi, ps):
                nc.vector.tensor_scalar_mul(q_t[oi][:, :], ps[:, :], 1.0 / FP8S)
            linear(wq, xh1, brows1, PROJ_Q, evict_q)

            k_t = [acts.tile([P, N], bf16, tag=f"k{i}", name=f"k{i}")
                   for i in range(CT)]

            def evict_k0(oi, ps):
                nc.vector.tensor_scalar_mul(k_t[oi][:, 0:T], ps[:, :],
                                            1.0 / FP8S)
            linear(wk, xh1, brows1, PROJ_K, evict_k0)

            xo_t = load_fm("xoT", "xo", bf16)
            bcA1o, brows1o = ln_stats(xo_t, oc_=ones_col_b)
            xh1o = ln_xhat_pool(xo_t, bcA1o, "xh1o")

            def evict_k1(oi, ps):
                nc.vector.tensor_scalar_mul(k_t[oi][:, T:N], ps[:, :],
                                            1.0 / FP8S)
            linear(wk, xh1o, brows1o, PROJ_K, evict_k1)

            c_t = load_fm("cT", "c", bf16)
            bcAc, browsc = ln_stats(c_t, oc_=ones_col_b)
            xhc = ln_xhat_pool(c_t, bcAc, "xhc")

            # ---------- V tiles (token-major interleaved fp8 pairs) ---------
            wv = load_w("WV8")
            v_tm = []
            for ktp in range(NKT // 2):
                vt = acts.tile([P, 2, H * VW], fp8, tag=f"vtm{ktp}",
                               name=f"vtm{ktp}")
                v_tm.append(vt)
                nc.sync.dma_start(
                    vt[:, :, :],
                    din["vinit"].rearrange("p (j w) -> p j w", j=2))

            def v_chunk(wv_, xh_, br_, uix, v_tiles, half, tj, oc):
                """One [128tok, 256feat] chunk of the V projection."""
                kt = half * 4 + tj
                ktp, par = kt // 2, kt % 2
                ps = ps_lin.tile([P, T], f32, tag="lin")
                for pi in range(CT // 2):
                    nc.tensor.matmul(
                        ps[:, 0:256],
                        xh_[:, 2 * pi:2 * pi + 2, tj * P:(tj + 1) * P],
                        wv_[:, 2 * pi:2 * pi + 2, oc * 256:(oc + 1) * 256],
                        start=(pi == 0), stop=False, perf_mode=DR)
                nc.tensor.matmul(
                    ps[:, 0:256], br_[0:1, :, tj * P:(tj + 1) * P],
                    urows[0:1, :,
                          uix * C + oc * 256:uix * C + (oc + 1) * 256],
                    start=False, stop=True, perf_mode=DR)
                vb = v_tiles[ktp][:, par, :]
                dst = bass.AP(vb.tensor, vb.offset + oc * 4 * VW,
                              [list(vb.ap[0]), [VW, 4], [1, 64]])
                nc.vector.tensor_scalar_mul(
                    dst, ps[:, 0:256].rearrange("p (h d) -> p h d", h=4),
                    1.0 / FP8S)

            # ---------- attention helpers (VW=128: ones cols -> rowsum) -----
            def attn_scores(q_tiles, k_tiles, h, nkt):
                th, ro = h // 2, 64 * (h % 2)
                ex_tiles = []
                for ktp in range(nkt // 2):
                    sps = ps_sc.tile([P, 2, T], f32, tag="score")
                    for par in range(2):
                        kt = 2 * ktp + par
                        nc.tensor.matmul(
                            sps[:, par, :],
                            k_tiles[th][ro:ro + 64, kt * P:(kt + 1) * P],
                            q_tiles[th][ro:ro + 64, :],
                            start=True, stop=True)
                    ex = expp.tile([P, 2, T], fp8, tag="exp")
                    nc.scalar.activation(ex[:, :, :], sps[:, :, :], AF.Exp,
                                         scale=float(D ** -0.5))
                    ex_tiles.append(ex)
                return ex_tiles

            def attn_av(ex_tiles, v_tiles, h, nkt, out_xh, gate_t=None):
                th, ro = h // 2, 64 * (h % 2)
                avps = ps_misc.tile([P, T], f32, tag="misc")
                for hcb in range(2):
                    cs = slice(hcb * HC, (hcb + 1) * HC)
                    for ktp in range(nkt // 2):
                        nc.tensor.matmul(
                            avps[:, cs],
                            v_tiles[ktp][:, :, h * VW:(h + 1) * VW],
                            ex_tiles[ktp][:, :, cs],
                            start=(ktp == 0), stop=(ktp == nkt // 2 - 1),
                            perf_mode=DR)
                rec = bcap.tile([64, T], bf16, tag="rec", bufs=2)
                nc.vector.reciprocal(rec[:, :], avps[64:128, :])
                dst = out_xh[ro:ro + 64, th, :]
                nc.vector.tensor_mul(dst, avps[0:64, :], rec[:, :])
                if gate_t is not None:
                    # gate_t holds 1 + tanh(z/2) = 2*sigmoid(z); the extra
                    # 2x is folded into W_proj_ca on the host
                    nc.gpsimd.tensor_mul(dst, dst, gate_t[th][ro:ro + 64, :])

            # ---------- SA attention interleaved with V / K2 / V2 -----------
            # per head: scores+exp (ACT-bound), PE fillers keep the tensor
            # engine busy; AV lags two heads so its exps are ready.
            sa_xh = acts.tile([P, CT, T], fp8, tag="sa_xh")
            wk2 = load_w("WK28")
            wv2 = load_w("WV28")
            k2_t = [acts.tile([P, T], bf16, tag=f"k2{i}", name=f"k2{i}")
                    for i in range(CT)]
            v2_tm = []
            for ktp in range(LKT // 2):
                vt = acts.tile([P, 2, H * VW], fp8, tag=f"v2tm{ktp}",
                               name=f"v2tm{ktp}")
                v2_tm.append(vt)
                nc.sync.dma_start(
                    vt[:, :, :],
                    din["vinit"].rearrange("p (j w) -> p j w", j=2))

            def evict_k2(oi, ps):
                nc.vector.tensor_scalar_mul(k2_t[oi][:, :], ps[:, :],
                                            1.0 / FP8S)

            sa_ex = {}
            for h in range(H):
                sa_ex[h] = attn_scores(q_t, k_t, h, NKT)
                # PE fillers
                if h < 6:
                    half, oc = h % 2, h // 2
                    xh_, br_ = (xh1, brows1) if half == 0 else (xh1o, brows1o)
                    for tj in range(4):
                        v_chunk(wv, xh_, br_, PROJ_V, v_tm, half, tj, oc)
                elif h == 6:
                    linear(wk2, xhc, browsc, PROJ_K2, evict_k2)
                else:
                    oc = h - 7
                    if oc < 3:
                        for tj in range(4):
                            v_chunk(wv2, xhc, browsc, PROJ_V2, v2_tm, 0, tj, oc)
                if h >= 2:
                    attn_av(sa_ex.pop(h - 2), v_tm, h - 2, NKT, sa_xh)
            for h in (H - 2, H - 1):
                attn_av(sa_ex.pop(h), v_tm, h, NKT, sa_xh)

            # ---------- proj_sa + gated residual ----------
            wp_ = load_w("WP8")

            def evict_res_gated(oi, ps):
                nc.vector.scalar_tensor_tensor(x_t[oi][:, :], ps[:, :],
                                               c_g64[:, oi:oi + 1],
                                               x_t[oi][:, :],
                                               ALU.mult, ALU.add)
            linear(wp_, sa_xh, brows1 if has_pbias else None,
                   PROJ_PSA if has_pbias else None, evict_res_gated)
            # x_t now holds x2

            # ---------- LN2 -> xh2; q2 / gate ----------
            bcA2, brows2 = ln_stats(x_t)
            xh2 = ln_xhat_dve(x_t, bcA2, "xh2")

            wqq = load_w("WQQ8")
            q2_t = [acts.tile([P, T], bf16, tag=f"q2{i}", name=f"q2{i}")
                    for i in range(CT)]

            def evict_q2(oi, ps):
                nc.vector.tensor_scalar_mul(q2_t[oi][:, :], ps[:, :], 1.0 / FP8S)
            linear(wqq, xh2, brows2, PROJ_QQ, evict_q2)

            wqg = load_w("WQG8")
            sig_t = [acts.tile([P, T], bf16, tag=f"sig{i}", name=f"sig{i}")
                     for i in range(CT)]

            def evict_sig(oi, ps):
                # tanh(z/2) lives in the exp act-table set (sigmoid does
                # not); sigmoid = (1 + tanh(z/2)) / 2 is finished in attn_av
                nc.scalar.activation(sig_t[oi][:, :], ps[:, :], AF.Tanh,
                                     scale=0.5 / FP8S)

            # ---------- CA attention (AV lags two heads) ----------
            # first two heads' scores go ahead of the gate sigmoids so the
            # ACT engine reaches the exp stream sooner
            ca_xh = acts.tile([P, CT, T], fp8, tag="ca_xh")
            ca_ex = {}
            for h in (0, 1):
                ca_ex[h] = attn_scores(q2_t, k2_t, h, LKT)
            linear(wqg, xh2, brows2, PROJ_QG, evict_sig)
            ones_bc = bcap.tile([P, T], bf16, tag="ones_bc")
            nc.vector.memset(ones_bc[:, :], 1.0)
            for oi in range(CT):
                # sig_t <- 1 + tanh(z/2) on Pool (keeps DVE off the AV path)
                nc.gpsimd.tensor_add(sig_t[oi][:, :], sig_t[oi][:, :],
                                     ones_bc[:, :])
            for h in range(2, H):
                ca_ex[h] = attn_scores(q2_t, k2_t, h, LKT)
                attn_av(ca_ex.pop(h - 2), v2_tm, h - 2, LKT, ca_xh,
                        gate_t=sig_t)
            for h in (H - 2, H - 1):
                attn_av(ca_ex.pop(h), v2_tm, h, LKT, ca_xh, gate_t=sig_t)

            # ---------- proj_ca + residual ----------
            wp2 = load_w("WP28")

            def evict_res(oi, ps):
                nc.vector.scalar_tensor_tensor(x_t[oi][:, :], ps[:, :],
                                               1.0 / FP8S, x_t[oi][:, :],
                                               ALU.mult, ALU.add)
                nc.sync.dma_start(x3_out[oi * P:(oi + 1) * P, :],
                                  x_t[oi][:, :])
            linear(wp2, ca_xh, brows1 if has_pbias else None,
                   PROJ_PCA if has_pbias else None, evict_res)
            # x_t now holds x3; LN3 / router / h2 all run on the host

    nc.finalize()
    return nc



# --------------------------------------------------------------------------
# Pass B kernel builder: fp8 DoubleRow expert FFN over token bins.
#
# Per core: len(binsizes) bins; bin b holds tokens of ONE (virtual) expert,
# whose pre-scaled fp8 weights stream in per bin. Tokens run in 512-wide
# matmul chunks (ap=512 keeps the PE sequencer off the critical path, vs
# ap=128 where Ldweights dispatch saturates PE.SEQ); h1 is produced in
# 3-plane PSUM groups (3 banks x 2 bufs) evicted by wide gelu ACT ops, and
# y accumulates per-oi in single-bank PSUM tiles evicted on DVE.
# --------------------------------------------------------------------------

FP8S = 64.0          # fp8 weight pre-scale (host multiplies W by this)
CHUNK = 512          # tokens per matmul chunk in pass B
GG = 3               # h1 psum group: 3 oj planes (3 banks; bufs=2 -> 6)
PASS_B_CONFIGS = [(640, 640, 384), (640, 640, 512), (768, 768, 512),
                  (1024, 1024, 1024)]


def _build_pass_b(binsizes, with_bias):
    nc = bacc.Bacc("TRN2", target_bir_lowering=False, debug=False, num_devices=8)
    fp8 = mybir.dt.float8e4
    DR = mybir.MatmulPerfMode.DoubleRow
    NB = len(binsizes)
    BSMAX = max(binsizes)

    h2b = nc.dram_tensor("h2b", [NB, P, CT * BSMAX], fp8, kind="ExternalInput")
    w1b = nc.dram_tensor("w1b", [NB, P, CT * F], fp8, kind="ExternalInput")
    w2b = nc.dram_tensor("w2b", [NB, P, FT * C], fp8, kind="ExternalInput")
    if with_bias:
        b1c = nc.dram_tensor("b1c", [NB, P, FT], f32, kind="ExternalInput")
        b2c = nc.dram_tensor("b2c", [NB, P, CT], f32, kind="ExternalInput")
    y_out = nc.dram_tensor("y", [NB, P, CT * BSMAX], bf16, kind="ExternalOutput")

    def chunks_of(BS):
        # even split into ceil(BS/CHUNK) chunks: avoids tiny ap tails
        # (e.g. 640 -> 320+320, not 512+128)
        n = -(-BS // CHUNK)
        base = BS // n
        rem = BS - base * n
        out = []
        t0 = 0
        for i in range(n):
            ch = base + (1 if i < rem else 0)
            out.append((t0, ch))
            t0 += ch
        return out

    with TileContext(nc) as tc:
        with tc.tile_pool(name="wp", bufs=2) as wp, \
             tc.tile_pool(name="hp", bufs=2) as hp, \
             tc.tile_pool(name="h1p", bufs=2) as h1p, \
             tc.tile_pool(name="yp", bufs=2) as yp, \
             tc.tile_pool(name="vec", bufs=2) as vecp, \
             tc.tile_pool(name="ps1", bufs=2, space="PSUM") as ps1p, \
             tc.tile_pool(name="psy", bufs=2, space="PSUM") as psyp:

            for b, BS in enumerate(binsizes):
                # h2 first (small), then w1 split per DoubleRow pair so the
                # first h1 matmuls start early; w2 streams under compute
                h2 = hp.tile([P, CT, BS], fp8, tag="h2")
                h2d_ = h2b[b].rearrange("p (k t) -> p k t", k=CT)
                nc.sync.dma_start(h2[:, 0:2, :], h2d_[:, 0:2, 0:BS])
                nc.sync.dma_start(h2[:, 2:CT, :], h2d_[:, 2:CT, 0:BS])
                w1 = wp.tile([P, CT, F], fp8, tag="w1")
                w1d = w1b[b].rearrange("p (k f) -> p k f", k=CT)
                for pi in range(CT // 2):
                    nc.sync.dma_start(w1[:, 2 * pi:2 * pi + 2, :],
                                      w1d[:, 2 * pi:2 * pi + 2, :])
                w2 = wp.tile([P, FT, C], fp8, tag="w2")
                w2d_ = w2b[b].rearrange("p (k f) -> p k f", k=FT)
                for wj in range(6):
                    nc.sync.dma_start(w2[:, 4 * wj:4 * (wj + 1), :],
                                      w2d_[:, 4 * wj:4 * (wj + 1), :])
                if with_bias:
                    b1 = vecp.tile([P, FT], f32, tag="b1")
                    nc.sync.dma_start(b1[:, :], b1c[b, :, :])
                    b2 = vecp.tile([P, CT], f32, tag="b2")
                    nc.sync.dma_start(b2[:, :], b2c[b, :, :])
                yt = yp.tile([P, CT, BS], bf16, tag="y")

                for t0, CH in chunks_of(BS):
                    rhs_h2 = h2[:, :, t0:t0 + CH]
                    h1 = h1p.tile([P, FT, CHUNK], fp8, tag="h1")
                    for g in range(FT // GG):
                        psh = ps1p.tile([P, GG, CHUNK], f32, tag="psh")
                        for oj in range(GG):
                            fo = (g * GG + oj) * P
                            for pi in range(CT // 2):
                                nc.tensor.matmul(
                                    psh[:, oj, 0:CH],
                                    w1[:, 2 * pi:2 * pi + 2, fo:fo + P],
                                    rhs_h2[:, 2 * pi:2 * pi + 2, :],
                                    start=(pi == 0), stop=(pi == CT // 2 - 1),
                                    perf_mode=DR)
                        dst = h1[:, g * GG:(g + 1) * GG, 0:CH]
                        if with_bias:
                            for oj in range(GG):
                                ojg = g * GG + oj
                                nc.scalar.activation(
                                    dst[:, oj, :], psh[:, oj, 0:CH], AF.Gelu,
                                    bias=b1[:, ojg:ojg + 1], scale=1.0 / FP8S)
                        else:
                            nc.scalar.activation(dst[:, :, :],
                                                 psh[:, :, 0:CH],
                                                 AF.Gelu, scale=1.0 / FP8S)
                    for oi in range(CT):
                        psy = psyp.tile([P, CHUNK], f32, tag="psy")
                        for pj in range(FT // 2):
                            nc.tensor.matmul(
                                psy[:, 0:CH],
                                w2[:, 2 * pj:2 * pj + 2, oi * P:(oi + 1) * P],
                                h1[:, 2 * pj:2 * pj + 2, 0:CH],
                                start=(pj == 0), stop=(pj == FT // 2 - 1),
                                perf_mode=DR)
                        ydst = yt[:, oi, t0:t0 + CH]
                        if with_bias:
                            nc.vector.tensor_scalar(
                                ydst, psy[:, 0:CH], 1.0 / FP8S,
                                b2[:, oi:oi + 1], ALU.mult, ALU.add)
                        else:
                            nc.vector.tensor_scalar_mul(ydst, psy[:, 0:CH],
                                                        1.0 / FP8S)
                # y export on the Pool DGE ring: keeps the SP ring free for
                # the next bin's weight stream (SP issues strictly in order,
                # so a compute-gated y DMA there would stall the prefetch)
                y_dst = y_out[b].rearrange("p (k t) -> p k t", k=CT)
                if BS > CHUNK:
                    nc.gpsimd.dma_start(y_dst[:, :, 0:BS - CHUNK],
                                        yt[:, :, 0:BS - CHUNK])
                    nc.gpsimd.dma_start(y_dst[:, :, BS - CHUNK:BS],
                                        yt[:, :, BS - CHUNK:BS])
                else:
                    nc.gpsimd.dma_start(y_dst[:, :, 0:BS], yt[:, :, 0:BS])

    nc.finalize()
    return nc


def _get_nc(which):
    if which not in _CACHE:
        if which[0] == "a":
            _CACHE[which] = _build_pass_a(has_pbias=which[1])
        else:
            _, binsizes, with_bias = which
            _CACHE[which] = _build_pass_b(binsizes, with_bias)
    return _CACHE[which]


# --------------------------------------------------------------------------
# Host orchestration
# --------------------------------------------------------------------------

def _silu(x):
    return x / (1.0 + np.exp(-x))


def _softmax(x, axis=-1):
    x = x - x.max(axis=axis, keepdims=True)
    e = np.exp(x)
    return e / e.sum(axis=axis, keepdims=True)


def _ln_np(v, eps=EPS):
    m = v.mean(-1, keepdims=True)
    var = v.var(-1, keepdims=True)
    return (v - m) / np.sqrt(var + eps)


def _refine_logits(logits, amb, x, c, mod_vecs, tcond, W_qkv, Wqq, Wqg,
                   W_kv, Wp_sa, bp_sa, Wp_ca, bp_ca, W_router):
    """Recompute router logits exactly (fp32 host) for ambiguous tokens.

    The device pass runs matmuls in float32r (~11-bit mantissa), which is
    enough to route every token whose top-2 margin exceeds ~1e-4. For the
    handful of near-tie tokens, redo the whole block math for just those
    tokens in fp32 so the expert choice matches a full-precision reference.
    """
    f = np.float32
    sh_msa, sc_msa, g_msa, sh_mlp, sc_mlp, g_mlp, gamma, beta = mod_vecs
    scale = f(D) ** -0.5
    for b_ in np.unique(amb // N):
        tloc = amb[amb // N == b_] % N
        hb = _ln_np(x[b_]) * (1.0 + sc_msa[b_]) + sh_msa[b_]      # [N, C]
        k = (hb @ W_qkv[:, C:2 * C]).reshape(N, H, D)
        v = (hb @ W_qkv[:, 2 * C:]).reshape(N, H, D)
        q = (hb[tloc] @ W_qkv[:, :C]).reshape(-1, H, D)
        s = np.einsum('ahd,lhd->ahl', q * scale, k)
        s = np.exp(s - s.max(-1, keepdims=True))
        attn = s / s.sum(-1, keepdims=True)
        sa = np.einsum('ahl,lhd->ahd', attn, v).reshape(-1, C)
        sa = sa @ Wp_sa + bp_sa
        x2a = x[b_, tloc] + g_msa[b_] * sa
        cm = _ln_np(c[b_]) * gamma[b_] + beta[b_]
        k2 = (cm @ W_kv[:, :C]).reshape(L, H, D)
        v2 = (cm @ W_kv[:, C:]).reshape(L, H, D)
        hxa = _ln_np(x2a)
        q2 = (hxa @ Wqq).reshape(-1, H, D)
        gate = (hxa @ Wqg).reshape(-1, H, D)
        s2 = np.einsum('ahd,lhd->ahl', q2 * scale, k2)
        s2 = np.exp(s2 - s2.max(-1, keepdims=True))
        attn2 = s2 / s2.sum(-1, keepdims=True)
        ao = np.einsum('ahl,lhd->ahd', attn2, v2)
        ao = ao * (1.0 / (1.0 + np.exp(-gate)))
        ca = ao.reshape(-1, C) @ Wp_ca + bp_ca
        x3a = x2a + ca
        h2a = _ln_np(x3a) * (1.0 + sc_mlp[b_]) + sh_mlp[b_]
        logits[b_ * N + tloc] = h2a @ W_router + tcond[b_]
    return logits


def kernel(x, c, t, W_ada, b_ada, W_qkv, W_proj_sa, b_proj_sa, W_q, W_kv,
           W_proj_ca, b_proj_ca, W_cadaln, b_cadaln, W_router, W_tcond,
           W1, b1, W2, b2, Ws1, bs1, Ws2, bs2):
    f = np.float32
    x, c, t = np.asarray(x, f), np.asarray(c, f), np.asarray(t, f)

    # ---- host: tiny t-conditioned vectors (per batch) ----
    st = _silu(t)
    mod = st @ np.asarray(W_ada, f) + np.asarray(b_ada, f)          # [B, 6C]
    sh_msa, sc_msa, g_msa, sh_mlp, sc_mlp, g_mlp = np.split(mod, 6, axis=-1)
    gb = st @ np.asarray(W_cadaln, f) + np.asarray(b_cadaln, f)     # [B, 2C]
    gamma, beta = np.split(gb, 2, axis=-1)
    tcond = t @ np.asarray(W_tcond, f)                              # [B, E]

    # ---- pass A inputs ----
    fp8np = ml_dtypes.float8_e4m3fn
    W_qkv = np.asarray(W_qkv, f)
    Wq_sa = np.ascontiguousarray(W_qkv[:, :C])
    Wk_sa = np.ascontiguousarray(W_qkv[:, C:2 * C])
    Wv_sa = np.ascontiguousarray(W_qkv[:, 2 * C:])
    W_q = np.asarray(W_q, f).reshape(C, H, 2 * D)
    Wqq = np.ascontiguousarray(W_q[:, :, :D].reshape(C, C))
    Wqg = np.ascontiguousarray(W_q[:, :, D:].reshape(C, C))
    W_kv = np.asarray(W_kv, f)
    Wk_ca = np.ascontiguousarray(W_kv[:, :C])
    Wv_ca = np.ascontiguousarray(W_kv[:, C:])
    Wp_sa = np.asarray(W_proj_sa, f)
    Wp_ca = np.asarray(W_proj_ca, f)
    bp_sa = np.asarray(b_proj_sa, f)
    bp_ca = np.asarray(b_proj_ca, f)
    W_router = np.asarray(W_router, f)
    has_pbias = bool(bp_sa.any() or bp_ca.any())
    global LAST_A_KEY
    LAST_A_KEY = ("a", has_pbias)

    onesr = np.ones((1, T), f)
    onesc = np.ones((P, 1), f)
    vinit = np.zeros((P, 2 * H * VW), fp8np)
    for blk in range(2 * H):
        vinit[:, blk * VW + 64:(blk + 1) * VW] = 1.0

    ONE = np.ones(C, f)
    batch_maps = []
    for b_ in range(B):
        sc1m = 1.0 + sc_msa[b_]
        gam = gamma[b_]
        Wqs = Wq_sa * sc1m[:, None]
        Wks = Wk_sa * sc1m[:, None]
        Wvs = Wv_sa * sc1m[:, None]
        Wk2s = Wk_ca * gam[:, None]
        Wv2s = Wv_ca * gam[:, None]
        urows = np.zeros((2, NPROJ * C), f)
        for uix, (u_, w0_) in {
            PROJ_Q: (ONE @ Wqs, sh_msa[b_] @ Wq_sa),
            PROJ_K: (ONE @ Wks, sh_msa[b_] @ Wk_sa),
            PROJ_V: (ONE @ Wvs, sh_msa[b_] @ Wv_sa),
            PROJ_QQ: (ONE @ Wqq, np.zeros(C, f)),
            PROJ_QG: (ONE @ Wqg, np.zeros(C, f)),
            PROJ_K2: (ONE @ Wk2s, beta[b_] @ Wk_ca),
            PROJ_V2: (ONE @ Wv2s, beta[b_] @ Wv_ca),
            PROJ_PSA: (np.zeros(C, f), bp_sa),
            PROJ_PCA: (np.zeros(C, f), bp_ca),
        }.items():
            urows[0, uix * C:(uix + 1) * C] = FP8S * u_
            urows[1, uix * C:(uix + 1) * C] = FP8S * w0_
        urows = urows.reshape(1, 2 * NPROJ * C).astype(fp8np)
        cols = np.ascontiguousarray((g_msa[b_] / FP8S).reshape(CT, P).T)
        batch_maps.append({
            "WQ8": prep_w(Wqs), "WK8": prep_w(Wks), "WV8": prep_w(Wvs),
            "WQQ8": prep_w(Wqq), "WQG8": prep_w(Wqg),
            "WK28": prep_w(Wk2s), "WV28": prep_w(Wv2s),
            # 0.5x: the CA gate is computed as (1 + tanh(z/2)) = 2*sigmoid(z)
            "WP8": prep_w(Wp_sa), "WP28": prep_w(0.5 * Wp_ca),
            "urows": urows, "cols": cols,
        })

    in_maps_a = []
    for core in range(8):
        b_, half = core // 2, core % 2
        sl = slice(half * T, (half + 1) * T)
        so = slice((1 - half) * T, (2 - half) * T)
        m = dict(batch_maps[b_])
        m.update({
            "xT": np.ascontiguousarray(x[b_, sl].T),
            "xoT": np.ascontiguousarray(x[b_, so].T).astype(ml_dtypes.bfloat16),
            "cT": np.ascontiguousarray(c[b_].T).astype(ml_dtypes.bfloat16),
            "onesr": onesr, "onesc": onesc, "vinit": vinit,
        })
        in_maps_a.append(m)

    nc_a = _get_nc(LAST_A_KEY)
    res_a = run_bass_kernel_spmd(nc_a, in_maps_a, core_ids=list(range(8)))

    x3 = np.empty((B, N, C), f)
    for core in range(8):
        b_, half = core // 2, core % 2
        sl = slice(half * T, (half + 1) * T)
        x3[b_, sl] = res_a.results[core]["x3T"].T

    # ---- host: LN3 + adaLN modulate + router (fp32) + top-2 ----
    xhat3 = _ln_np(x3)                                         # [B, N, C]
    h2_full = xhat3 * (1.0 + sc_mlp[:, None, :]) + sh_mlp[:, None, :]
    logits = (h2_full @ W_router + tcond[:, None, :]).reshape(B * N, E)
    h2pm_full = np.ascontiguousarray(
        h2_full.reshape(B * N, CT, P).transpose(2, 1, 0)).astype(fp8np)
    probs = _softmax(logits, axis=-1)
    # near-tie tokens: fp8/f32r rounding on device could flip their top-2
    # choice vs a full-precision reference -- redo those on host in fp32
    ps_sorted = np.sort(probs, axis=-1)
    amb = np.nonzero(ps_sorted[:, -2] - ps_sorted[:, -3] < 1e-2)[0]
    if len(amb):
        mod_vecs = (sh_msa, sc_msa, g_msa, sh_mlp, sc_mlp, g_mlp, gamma, beta)
        logits = _refine_logits(logits, amb, x, c, mod_vecs, tcond, W_qkv,
                                Wqq, Wqg, W_kv, Wp_sa, bp_sa, Wp_ca, bp_ca,
                                W_router)
        probs[amb] = _softmax(logits[amb], axis=-1)
    order = np.argsort(-probs, axis=-1, kind="stable")
    topi = order[:, :TOPK]
    topv = np.take_along_axis(probs, topi, axis=-1)
    topv = topv / topv.sum(-1, keepdims=True)

    W1 = np.asarray(W1, f)
    W2 = np.asarray(W2, f)
    b1 = np.asarray(b1, f)
    b2 = np.asarray(b2, f)
    Ws1 = np.asarray(Ws1, f)
    Ws2 = np.asarray(Ws2, f)
    bs1 = np.asarray(bs1, f)
    bs2 = np.asarray(bs2, f)
    with_bias = bool(b1.any() or b2.any() or bs1.any() or bs2.any())

    # ---- bin packing: 8 cores x NB bins; each bin = tokens of one expert ----
    # expert e token list (order arbitrary), shared pseudo-expert = E
    tok_by_e = [np.nonzero(topi == e_)[0] for e_ in range(E)]
    wv_by_e = [topv[topi == e_] for e_ in range(E)]
    all_toks = np.arange(B * N)
    tok_by_e.append(all_toks)
    wv_by_e.append(np.ones(B * N, f))

    def pack(binsizes):
        """Assign expert pieces to the 8*len(binsizes) bins. Per-expert
        knapsack over bin-size counts with backtracking; shared expert
        fills whatever remains. Returns per-bin (expert, toks, wv) or None."""
        import itertools as _it
        nbins = 8 * len(binsizes)
        bin_sz = [binsizes[i % len(binsizes)] for i in range(nbins)]
        sizes = sorted(set(bin_sz), reverse=True)
        avail0 = tuple(sum(1 for s in bin_sz if s == sz) for sz in sizes)
        order = sorted(range(E), key=lambda e_: -len(tok_by_e[e_]))
        items = [len(tok_by_e[e_]) for e_ in order]
        shared_n = B * N

        def options(cnt, avail):
            opts = []
            maxn = [min(a, cnt // s + 2) for s, a in zip(sizes, avail)]
            for combo in _it.product(*[range(m + 1) for m in maxn]):
                tot = sum(n * s for n, s in zip(combo, sizes))
                if tot >= cnt and tot - cnt < sizes[-1]:
                    opts.append(combo)
            opts.sort(key=lambda c: sum(n * s for n, s in zip(c, sizes)))
            return opts[:40]

        def rec(idx, avail):
            if idx == len(items):
                if sum(a * s for a, s in zip(avail, sizes)) >= shared_n:
                    return []
                return None
            for combo in options(items[idx], avail):
                if all(n <= a for n, a in zip(combo, avail)):
                    sub = rec(idx + 1,
                              tuple(a - n for a, n in zip(avail, combo)))
                    if sub is not None:
                        return [combo] + sub
            return None

        combos = rec(0, avail0)
        if combos is None:
            return None
        # materialize: free bin ids per size
        free = {sz: [i for i in range(nbins) if bin_sz[i] == sz]
                for sz in sizes}
        assign = [None] * nbins
        for e_, combo in zip(order, combos):
            toks, wv = tok_by_e[e_], wv_by_e[e_]
            pos = 0
            for sz, n in zip(sizes, combo):
                for _ in range(n):
                    bid = free[sz].pop()
                    take = min(len(toks) - pos, sz)
                    if take > 0:
                        assign[bid] = (e_, toks[pos:pos + take],
                                       wv[pos:pos + take])
                        pos += take
        rem_bins = [i for sz in sizes for i in free[sz]]
        toks, wv = tok_by_e[E], wv_by_e[E]
        pos = 0
        for bid in rem_bins:
            take = min(len(toks) - pos, bin_sz[bid])
            if take > 0:
                assign[bid] = (E, toks[pos:pos + take], wv[pos:pos + take])
                pos += take
        if pos < shared_n:
            return None
        return assign

    assign = None
    for cfg in PASS_B_CONFIGS:
        assign = pack(list(cfg))
        if assign is not None:
            binsizes = list(cfg)
            break
    assert assign is not None, "no pass-B config fits this routing"
    global LAST_B_KEY
    LAST_B_KEY = ("b", tuple(binsizes), with_bias)

    NB = len(binsizes)
    BSMAX = max(binsizes)

    w1_pre = [prep_w(W1[e_]) for e_ in range(E)] + [prep_w(Ws1)]
    w2_pre = [prep_w(W2[e_]) for e_ in range(E)] + [prep_w(Ws2)]
    b1_all = np.concatenate([b1, bs1[None]], 0)   # [E+1, F]
    b2_all = np.concatenate([b2, bs2[None]], 0)   # [E+1, C]

    # h2 tokens in fp8 partition-major from the host LN3: h2pm_full [P,CT,BN]
    h2pm = h2pm_full

    in_maps_b = []
    for core in range(8):
        h2bin = np.zeros((NB, P, CT * BSMAX), fp8np)
        w1bin = np.empty((NB, P, CT * F), fp8np)
        w2bin = np.empty((NB, P, FT * C), fp8np)
        m = {"h2b": h2bin, "w1b": w1bin, "w2b": w2bin}
        if with_bias:
            m["b1c"] = np.zeros((NB, P, FT), f)
            m["b2c"] = np.zeros((NB, P, CT), f)
        for s in range(NB):
            a = assign[core * NB + s]
            BS = binsizes[s]
            if a is None:
                w1bin[s] = 0
                w2bin[s] = 0
                continue
            e_, toks, _wv = a
            w1bin[s] = w1_pre[e_]
            w2bin[s] = w2_pre[e_]
            h2bin[s].reshape(P, CT, BSMAX)[:, :, :len(toks)] = h2pm[:, :, toks]
            if with_bias:
                m["b1c"][s] = b1_all[e_].reshape(FT, P).T
                m["b2c"][s] = b2_all[e_].reshape(CT, P).T
        in_maps_b.append(m)

    nc_b = _get_nc(LAST_B_KEY)
    res_b = run_bass_kernel_spmd(nc_b, in_maps_b, core_ids=list(range(8)))

    # ---- host: weighted scatter-add combine + final residual ----
    accum = np.zeros((B * N, C), f)
    for core in range(8):
        y = np.asarray(res_b.results[core]["y"], f)  # [NB, P, CT*BSMAX]
        for s in range(NB):
            a = assign[core * NB + s]
            if a is None:
                continue
            e_, toks, wv = a
            # y[p, k, t] = out feature k*P+p of token t
            yv = y[s].reshape(P, CT, BSMAX)[:, :, :len(toks)]
            accum[toks] += wv[:, None] * yv.transpose(2, 1, 0).reshape(-1, C)

    out = x3 + g_mlp[:, None, :] * accum.reshape(B, N, C)
    return out.astype(np.float32)

